# revision 9
# baseline (speedup 1.0000x reference)
"""MoE gate (DeepSeek-V2 style, group-limited greedy top-k) for Trainium2.

Full-input contract: kernel(hidden_states[4,8192,2048] f32, kernel[64,2048] f32)
-> topk_weight [32768, 6] f32.

Strategy: pure data-parallel over 8 NeuronCores (4096 tokens each).

Numerics: x is decomposed on-chip as x = a + r with a = f16(x) and
r = f16(x - a) (engine copy + subtract; the rounding mode of a is irrelevant
because r compensates exactly); w is decomposed once as w = u + s the same
way. Then three f16 matmul passes

    logitsT += uT.aT + uT.rT + sT.aT        (skipped term rT.sT ~ 2^-22)

reproduce fp32 logits to ~4e-6 absolute (measured on HW), far below the
~1e-5 group-selection margin of this input, while running at 1 cycle/row on
the PE -- and, unlike the f32r hi/lo scheme, the operands are 2-byte so the
transposition moves off the PE entirely:

Transposition: every [128t x 128h] chunk of a and r is transposed
SBUF->SBUF by the XBAR DMA-transpose engine on the SP/ACT HWDGE queues
(~56 ns of queue time per chunk in the cost model), eliminating both the PE
transpose passes and the PSUM->SBUF evictions of the f32r design.
(The gpsimd cast-DMA (f32->f16 straight from DRAM) would be cheaper still,
but on real HW an XBAR transpose reading an SBUF region written by a SWDGE
DMA races with it -- measured data corruption -- so a is engine-computed.)

Per core: tokens are remapped so partition p owns a contiguous 32-token DRAM
range (t = p*32 + m*4 + b), making every DMA descriptor large & contiguous.
Issue order is software-pipelined at megatile granularity:
fronts(m) [loads + a/r ops] -> matmuls(m-1) -> backs(m) [transposes] ->
post(m-1) [lts, logits back-transpose, top-k, store], so no engine FIFO
blocks on a dependency issued just before it.

Top-k per 128-token block on DVE/Pool/ACT using the hardware top-8 sort
(InstMax): softmax denominator cancels in the final normalization, so only
e = exp(logit - max) is needed; group-max -> sort -> 3rd value threshold ->
group mask -> masked e -> top-8 sort -> sum top-6 -> reciprocal -> scale.

Engine budget per core (cost model, 4096 tokens):
  PE   48 f16 mm/megatile + lg transposes + warmup ~87 us   <- critical
  SP   16 f32 loads + half the transposes + stores ~82 us
  ACT  24 a-ops + half transposes + exp/lts        ~84 us
  Pool 16 f32 loads + 4 a + 4 r + lsub/me          ~87 us
  DVE  28 r + 4 a + rest of top-k                  ~85 us
"""

import sys

if "/opt/trn_rl_repo" not in sys.path:
    sys.path.insert(0, "/opt/trn_rl_repo")

import numpy as np

# Problem constants (hardcoded per contract)
N_CORES = 8
H = 2048
E = 64  # n_routed_experts
G = 8  # n_group
PG = E // G  # experts per group
TG = 3  # topk_group
TK = 6  # top_k
P = 128  # partitions
MEGA = 512  # tokens per megatile
BB = MEGA // P  # 4 token blocks per megatile
KCH = H // P  # 16 contraction chunks


def build_nc(t_core, repeat=1):
    """Build the single-core Bass program for a t_core-token shard."""
    from concourse import bacc, mybir, masks
    from concourse.tile import TileContext

    f32 = mybir.dt.float32
    f16 = mybir.dt.float16
    X = mybir.AxisListType.X
    NM = t_core // MEGA
    assert t_core % MEGA == 0

    nc = bacc.Bacc()
    x = nc.declare_dram_parameter("x", [t_core, H], f32, isOutput=False)
    w = nc.declare_dram_parameter("w", [E, H], f32, isOutput=False)
    out = nc.declare_dram_parameter("out", [t_core, TK], f32, isOutput=True)

    with TileContext(nc) as tc:
        with (
            tc.tile_pool(name="const", bufs=1) as cpool,
            tc.tile_pool(name="xq", bufs=5) as xqpool,
            tc.tile_pool(name="aq", bufs=6) as aqpool,
            tc.tile_pool(name="at", bufs=2) as atpool,
            tc.tile_pool(name="rt", bufs=2) as rtpool,
            tc.tile_pool(name="lts", bufs=2) as ltspool,
            tc.tile_pool(name="small", bufs=2) as spool,
            tc.tile_pool(name="outp", bufs=2) as opool,
            tc.tile_pool(name="ps_mm", bufs=2, space="PSUM") as psmm,
            tc.tile_pool(name="ps_lg", bufs=2, space="PSUM") as pslg,
            tc.tile_pool(name="ps_wm", bufs=1, space="PSUM") as pswm,
        ):
            identf = cpool.tile([P, P], f32)
            masks.make_identity(nc, identf[:])
            idf = identf[:]

            w_sb = cpool.tile([E, H], f32)
            u_sb = cpool.tile([E, H], f16)
            s_sb = cpool.tile([E, H], f16)
            uT = cpool.tile([P, KCH, E], f16)
            sT = cpool.tile([P, KCH, E], f16)

            def warm_pe(n=26):
                # Dummy identity transposes burn through the PE p-state ramp
                # (~3us of continuous activity) during the otherwise PE-idle
                # DMA head, so real matmuls start at full clock.
                pwm = pswm.tile([P, P], f32, tag="wm")
                for _ in range(n):
                    nc.tensor.transpose(pwm[:], idf, idf)

            def setup_w():
                # w -> u = f16(w), s = f16(w - u); DMA-transpose both to
                # [128h, k, 64e]. Issued after megatile 0's fronts so it
                # doesn't gate the pipeline head.
                nc.sync.dma_start(out=w_sb[:], in_=w[:])
                nc.scalar.copy(u_sb[:], w_sb[:])
                nc.vector.tensor_tensor(
                    s_sb[:], w_sb[:], u_sb[:], mybir.AluOpType.subtract
                )
                for k in range(KCH):
                    nc.scalar.dma_start(
                        out=uT[:, k, :],
                        in_=u_sb[:, k * P : (k + 1) * P],
                        transpose=True,
                    )
                    nc.scalar.dma_start(
                        out=sT[:, k, :],
                        in_=s_sb[:, k * P : (k + 1) * P],
                        transpose=True,
                    )

            xr = x[:].rearrange("(p m b) h -> p m b h", p=P, m=NM, b=BB)
            our = out[:].rearrange("(p m b) k -> p m b k", p=P, m=NM, b=BB)

            def front_megatile(m):
                # Per quarter b: f32 load (SP / Pool SWDGE), a = f16(x)
                # (ACT / DVE / Pool), r = f16(x - a) (DVE / Pool), both into
                # one combined arq tile. SP's DMA ring carries ONLY plain
                # copies and ACT's ring ONLY XBAR transposes: mixing the two
                # modes in one ring corrupts in-flight descriptors on real
                # HW (no xbar_mode serialization in this bass version).
                quarters = []
                for b in range(BB):
                    xt = xqpool.tile([P, H], f32, tag="xq")
                    if b < 3 or m % 4 < 1:
                        load_eng = nc.sync
                    else:
                        load_eng = nc.gpsimd
                    load_eng.dma_start(out=xt[:], in_=xr[:, m, b, :])
                    arq = aqpool.tile([P, 2, H], f16, tag="arq")
                    if b == 0 or (b == 1 and m % 2 == 0):
                        nc.scalar.copy(arq[:, 0, :], xt[:])
                    elif b == 1 or b == 2:
                        nc.gpsimd.tensor_copy(arq[:, 0, :], xt[:])
                    else:
                        nc.gpsimd.tensor_copy(arq[:, 0, :], xt[:])
                    r_eng = nc.gpsimd if (b == 3 and m == 7) else nc.vector
                    r_eng.tensor_tensor(
                        arq[:, 1, :], xt[:], arq[:, 0, :],
                        mybir.AluOpType.subtract,
                    )
                    quarters.append(arq)
                return quarters

            def back_megatile(m, quarters):
                # 32 XBAR DMA transposes per quarter into aT/rT
                # [128h, k, 512t], all on the ACT HWDGE ring (kept pure
                # transpose-mode). A dummy transpose of the arq head acts as
                # a fence: it waits on the a/r producer semaphores and its
                # pipeline latency lets the engine writes settle in SBUF
                # before the real XBAR reads start.
                at = atpool.tile([P, KCH, MEGA], f16, tag="at")
                rt = rtpool.tile([P, KCH, MEGA], f16, tag="rt")
                for b, arq in enumerate(quarters):
                    fd = spool.tile([P, P], f16, tag="fd")
                    nc.scalar.dma_start(
                        out=fd[:], in_=arq[:, 1, 0:P], transpose=True
                    )
                    for k in range(KCH):
                        nc.scalar.dma_start(
                            out=at[:, k, b * P : (b + 1) * P],
                            in_=arq[:, 0, k * P : (k + 1) * P],
                            transpose=True,
                        )
                        nc.scalar.dma_start(
                            out=rt[:, k, b * P : (b + 1) * P],
                            in_=arq[:, 1, k * P : (k + 1) * P],
                            transpose=True,
                        )
                return at, rt

            def compute_mm(at, rt, t0=0, width=MEGA):
                # logitsT[64, width] += uT.aT + uT.rT + sT.aT (f16, 1 cyc/row)
                lt = psmm.tile([E, width], f32, tag="lt")
                n_acc = 3 * KCH
                i_acc = 0
                for k in range(KCH):
                    for wt_k, xt_k in ((uT, at), (uT, rt), (sT, at)):
                        nc.tensor.matmul(
                            lt[:],
                            wt_k[:, k, :],
                            xt_k[:, k, t0 : t0 + width],
                            start=(i_acc == 0),
                            stop=(i_acc == n_acc - 1),
                        )
                        i_acc += 1
                return lt

            def compute_post(m, lt, t0=0, width=MEGA):
                nb = width // P  # token blocks in this slice
                b0 = t0 // P
                lts = ltspool.tile([E, width], f32, tag="lts")
                nc.scalar.copy(lts[:], lt[:])

                # transpose logits back -> [128t, 64e] blocks in PSUM (fp32)
                lg = pslg.tile([P, nb * E], f32, tag="lg")
                for b in range(nb):
                    nc.tensor.transpose(
                        lg[:, b * E : (b + 1) * E],
                        lts[:, b * P : (b + 1) * P],
                        idf[0:E, 0:E],
                    )

                # --- top-k pipeline, all nb token-blocks fused per op ---
                lg3 = lg[:].rearrange("p (b e) -> p b e", b=nb)
                # e = exp(logit - max): keeps ACT exp args in [-24, 0] where
                # the table is ~4x more accurate (fewer selection-flip risks
                # near group-boundary ties). Per-block bias via DVE subtract.
                nmax = spool.tile([P, nb], f32, tag="nmax")
                nc.vector.tensor_reduce(
                    nmax[:], lg3, axis=X, op=mybir.AluOpType.max, negate=True
                )
                lsub = spool.tile([P, nb, E], f32, tag="lsub")
                nc.vector.tensor_tensor(
                    lsub[:],
                    lg3,
                    nmax[:].unsqueeze(2).broadcast_to([P, nb, E]),
                    mybir.AluOpType.add,
                )
                e_sb = spool.tile([P, nb, E], f32, tag="esb")
                nc.scalar.activation(
                    e_sb[:], lsub[:], mybir.ActivationFunctionType.Exp
                )
                e4 = e_sb[:].rearrange("p b (g j) -> p b g j", g=G)
                gmax = spool.tile([P, nb, G], f32, tag="gmax")
                nc.vector.tensor_reduce(
                    gmax[:], e4, axis=X, op=mybir.AluOpType.max
                )
                gsort = spool.tile([P, nb, 8], f32, tag="gsort")
                for b in range(nb):
                    nc.vector.max(gsort[:, b, :], gmax[:, b, :])
                gmask = spool.tile([P, nb, G], f32, tag="gmask")
                nc.vector.tensor_tensor(
                    gmask[:],
                    gmax[:],
                    gsort[:, :, TG - 1 : TG].broadcast_to([P, nb, G]),
                    mybir.AluOpType.is_ge,
                )
                me = spool.tile([P, nb, E], f32, tag="me")
                nc.gpsimd.tensor_tensor(
                    me[:].rearrange("p b (g j) -> p b g j", g=G),
                    e4,
                    gmask[:].unsqueeze(3).broadcast_to([P, nb, G, PG]),
                    mybir.AluOpType.mult,
                )
                t8 = spool.tile([P, nb, 8], f32, tag="t8")
                for b in range(nb):
                    nc.vector.max(t8[:, b, :], me[:, b, :])
                ssum = spool.tile([P, nb], f32, tag="ssum")
                nc.vector.tensor_reduce(
                    ssum[:], t8[:, :, 0:TK], axis=X, op=mybir.AluOpType.add
                )
                rec = spool.tile([P, nb], f32, tag="rec")
                nc.vector.reciprocal(rec[:], ssum[:])
                ow = opool.tile([P, nb, TK], f32, tag="ow")
                nc.vector.tensor_tensor(
                    ow[:],
                    t8[:, :, 0:TK],
                    rec[:].unsqueeze(2).broadcast_to([P, nb, TK]),
                    mybir.AluOpType.mult,
                )
                nc.sync.dma_start(out=our[:, m, b0 : b0 + nb], in_=ow[:])

            # software pipeline:
            #   fronts(m) -> matmuls(m-1) -> backs(m) -> post(m-1)
            prev = None  # (m, lt PSUM tile)
            prev_t = None  # (at, rt) awaiting matmul
            w_done = False
            warm_pe()
            for _r in range(repeat):
                for m in range(NM):
                    quarters = front_megatile(m)
                    if not w_done:
                        setup_w()
                        w_done = True
                    if prev_t is not None:
                        lt = compute_mm(*prev_t[1:])
                        prev = (prev_t[0], lt)
                    cur_t = (m, *back_megatile(m, quarters))
                    if prev is not None:
                        compute_post(*prev)
                        prev = None
                    prev_t = cur_t
            if prev_t is not None:
                # split the final megatile so its top-k overlaps the second
                # half-chain instead of serializing after the last matmul
                m_l, at_l, rt_l = prev_t
                lt1 = compute_mm(at_l, rt_l, 0, MEGA // 2)
                compute_post(m_l, lt1, 0, MEGA // 2)
                lt2 = compute_mm(at_l, rt_l, MEGA // 2, MEGA // 2)
                compute_post(m_l, lt2, MEGA // 2, MEGA // 2)

    nc.compile()
    return nc


_NC_CACHE = {}


def _get_nc(t_core):
    if t_core not in _NC_CACHE:
        _NC_CACHE[t_core] = build_nc(t_core)
    return _NC_CACHE[t_core]


def run_sharded(flat_x, w, trace=False, **kw):
    """flat_x: [T, H] f32. Returns ([T, 6] f32, BassKernelResults)."""
    from concourse.bass_utils import run_bass_kernel_spmd

    T = flat_x.shape[0]
    tc = T // N_CORES
    nc = _get_nc(tc)
    in_maps = [
        {"x": np.ascontiguousarray(flat_x[i * tc : (i + 1) * tc]), "w": w}
        for i in range(N_CORES)
    ]
    res = run_bass_kernel_spmd(nc, in_maps, list(range(N_CORES)), trace=trace, **kw)
    outs = [np.asarray(res.results[i]["out"]) for i in range(N_CORES)]
    return np.concatenate(outs, axis=0), res


def kernel(hidden_states, kernel):
    hs = np.asarray(hidden_states, dtype=np.float32)
    w = np.ascontiguousarray(np.asarray(kernel, dtype=np.float32))
    B, S, Hh = hs.shape
    flat = np.ascontiguousarray(hs.reshape(B * S, Hh))
    out, _ = run_sharded(flat, w)
    return out


# revision 10
# speedup vs baseline: 1.1200x; 1.1200x over previous
"""MoE gate (DeepSeek-V2 style, group-limited greedy top-k) for Trainium2.

Full-input contract: kernel(hidden_states[4,8192,2048] f32, kernel[64,2048] f32)
-> topk_weight [32768, 6] f32.

Strategy: pure data-parallel over 8 NeuronCores (4096 tokens each).

Numerics: x is decomposed on-chip as x = a + r with a = f16(x) and
r = f16(x - a) (engine copy + subtract; the rounding mode of a is irrelevant
because r compensates exactly); w is decomposed once as w = u + s the same
way. Then three f16 matmul passes

    logitsT += uT.aT + uT.rT + sT.aT        (skipped term rT.sT ~ 2^-22)

reproduce fp32 logits to ~4e-6 absolute (measured on HW), far below the
~1e-5 group-selection margin of this input, while running at 1 cycle/row on
the PE -- and, unlike the f32r hi/lo scheme, the operands are 2-byte so the
transposition moves off the PE entirely:

Transposition: every [128t x 128h] chunk of a and r is transposed
SBUF->SBUF by the XBAR DMA-transpose engine on the SP/ACT HWDGE queues
(~56 ns of queue time per chunk in the cost model), eliminating both the PE
transpose passes and the PSUM->SBUF evictions of the f32r design.
(The gpsimd cast-DMA (f32->f16 straight from DRAM) would be cheaper still,
but on real HW an XBAR transpose reading an SBUF region written by a SWDGE
DMA races with it -- measured data corruption -- so a is engine-computed.)

Per core: tokens are remapped so partition p owns a contiguous 32-token DRAM
range (t = p*32 + m*4 + b), making every DMA descriptor large & contiguous.
Issue order is software-pipelined at megatile granularity:
fronts(m) [loads + a/r ops] -> matmuls(m-1) -> backs(m) [transposes] ->
post(m-1) [lts, logits back-transpose, top-k, store], so no engine FIFO
blocks on a dependency issued just before it.

Top-k per 128-token block on DVE/Pool/ACT using the hardware top-8 sort
(InstMax): softmax denominator cancels in the final normalization, so only
e = exp(logit - max) is needed; group-max -> sort -> 3rd value threshold ->
group mask -> masked e -> top-8 sort -> sum top-6 -> reciprocal -> scale.

Engine budget per core (cost model, 4096 tokens):
  PE   48 f16 mm/megatile + lg transposes + warmup ~87 us   <- critical
  SP   16 f32 loads + half the transposes + stores ~82 us
  ACT  24 a-ops + half transposes + exp/lts        ~84 us
  Pool 16 f32 loads + 4 a + 4 r + lsub/me          ~87 us
  DVE  28 r + 4 a + rest of top-k                  ~85 us
"""

import sys

if "/opt/trn_rl_repo" not in sys.path:
    sys.path.insert(0, "/opt/trn_rl_repo")

import numpy as np

# Problem constants (hardcoded per contract)
N_CORES = 8
H = 2048
E = 64  # n_routed_experts
G = 8  # n_group
PG = E // G  # experts per group
TG = 3  # topk_group
TK = 6  # top_k
P = 128  # partitions
MEGA = 512  # tokens per megatile
BB = MEGA // P  # 4 token blocks per megatile
KCH = H // P  # 16 contraction chunks


def build_nc(t_core, repeat=1):
    """Build the single-core Bass program for a t_core-token shard."""
    from concourse import bacc, mybir, masks
    from concourse.tile import TileContext

    f32 = mybir.dt.float32
    f16 = mybir.dt.float16
    X = mybir.AxisListType.X
    NM = t_core // MEGA
    assert t_core % MEGA == 0

    nc = bacc.Bacc()
    x = nc.declare_dram_parameter("x", [t_core, H], f32, isOutput=False)
    w = nc.declare_dram_parameter("w", [E, H], f32, isOutput=False)
    out = nc.declare_dram_parameter("out", [t_core, TK], f32, isOutput=True)

    with TileContext(nc) as tc:
        with (
            tc.tile_pool(name="const", bufs=1) as cpool,
            tc.tile_pool(name="xq", bufs=5) as xqpool,
            tc.tile_pool(name="aq", bufs=6) as aqpool,
            tc.tile_pool(name="at", bufs=2) as atpool,
            tc.tile_pool(name="rt", bufs=2) as rtpool,
            tc.tile_pool(name="lts", bufs=2) as ltspool,
            tc.tile_pool(name="small", bufs=2) as spool,
            tc.tile_pool(name="outp", bufs=2) as opool,
            tc.tile_pool(name="ps_mm", bufs=2, space="PSUM") as psmm,
            tc.tile_pool(name="ps_lg", bufs=2, space="PSUM") as pslg,
            tc.tile_pool(name="ps_wm", bufs=1, space="PSUM") as pswm,
        ):
            identf = cpool.tile([P, P], f32)
            masks.make_identity(nc, identf[:])
            idf = identf[:]

            w_sb = cpool.tile([E, H], f32)
            u_sb = cpool.tile([E, H], f16)
            s_sb = cpool.tile([E, H], f16)
            uT = cpool.tile([P, KCH, E], f16)
            sT = cpool.tile([P, KCH, E], f16)

            def warm_pe(n=26):
                # Dummy identity transposes burn through the PE p-state ramp
                # (~3us of continuous activity) during the otherwise PE-idle
                # DMA head, so real matmuls start at full clock.
                pwm = pswm.tile([P, P], f32, tag="wm")
                for _ in range(n):
                    nc.tensor.transpose(pwm[:], idf, idf)

            def setup_w():
                # w -> u = f16(w), s = f16(w - u); DMA-transpose both to
                # [128h, k, 64e]. Issued after megatile 0's fronts so it
                # doesn't gate the pipeline head.
                nc.gpsimd.dma_start(out=w_sb[:], in_=w[:])
                nc.scalar.copy(u_sb[:], w_sb[:])
                nc.vector.tensor_tensor(
                    s_sb[:], w_sb[:], u_sb[:], mybir.AluOpType.subtract
                )
                for k in range(KCH):
                    nc.scalar.dma_start(
                        out=uT[:, k, :],
                        in_=u_sb[:, k * P : (k + 1) * P],
                        transpose=True,
                    )
                    nc.scalar.dma_start(
                        out=sT[:, k, :],
                        in_=s_sb[:, k * P : (k + 1) * P],
                        transpose=True,
                    )

            xr = x[:].rearrange("(p m b) h -> p m b h", p=P, m=NM, b=BB)
            our = out[:].rearrange("(p m b) k -> p m b k", p=P, m=NM, b=BB)

            def front_megatile(m):
                # Per quarter b: f32 load (all on the Pool SWDGE ring -- the
                # SP/ACT HWDGE rings carry ONLY XBAR transposes: mixing
                # transpose- and copy-mode descriptors in one ring corrupts
                # data on real HW), a = f16(x) (ACT, some DVE), r = f16(x-a)
                # (DVE), into one combined arq tile.
                quarters = []
                for b in range(BB):
                    xt = xqpool.tile([P, H], f32, tag="xq")
                    nc.gpsimd.dma_start(out=xt[:], in_=xr[:, m, b, :])
                    arq = aqpool.tile([P, 2, H], f16, tag="arq")
                    if b < 3 or m % 8 < 3:
                        nc.scalar.copy(arq[:, 0, :], xt[:])
                    else:
                        nc.vector.tensor_copy(arq[:, 0, :], xt[:])
                    nc.vector.tensor_tensor(
                        arq[:, 1, :], xt[:], arq[:, 0, :],
                        mybir.AluOpType.subtract,
                    )
                    quarters.append(arq)
                return quarters

            def back_megatile(m, quarters):
                # XBAR transposes into aT (SP ring) and rT (rt1 on SP for
                # quarter 0, rt2 on ACT for quarters 1-3); each destination
                # tile has a single writing ring. A dummy transpose of the
                # arq head fences each quarter: it waits on the a/r producer
                # semaphores and its pipeline latency lets the engine writes
                # settle before the real XBAR reads.
                at = atpool.tile([P, KCH, MEGA], f16, tag="at")
                rt1 = rtpool.tile([P, KCH, P], f16, tag="rt1")
                rt2 = rtpool.tile([P, KCH, 3 * P], f16, tag="rt2")
                for b, arq in enumerate(quarters):
                    fd = spool.tile([P, P], f16, tag=f"fd{min(b, 1)}")
                    f_eng = nc.sync if b == 0 else nc.scalar
                    f_eng.dma_start(
                        out=fd[:], in_=arq[:, 1, 0:P], transpose=True
                    )
                    for k in range(KCH):
                        nc.sync.dma_start(
                            out=at[:, k, b * P : (b + 1) * P],
                            in_=arq[:, 0, k * P : (k + 1) * P],
                            transpose=True,
                        )
                        if b == 0:
                            nc.sync.dma_start(
                                out=rt1[:, k, :],
                                in_=arq[:, 1, k * P : (k + 1) * P],
                                transpose=True,
                            )
                        else:
                            nc.scalar.dma_start(
                                out=rt2[:, k, (b - 1) * P : b * P],
                                in_=arq[:, 1, k * P : (k + 1) * P],
                                transpose=True,
                            )
                return at, (rt1, rt2)

            def compute_mm(at, rt, t0=0, width=MEGA):
                # logitsT[64, width] += uT.aT + uT.rT + sT.aT (f16, 1 cyc/row)
                rt1, rt2 = rt
                lt = psmm.tile([E, width], f32, tag="lt")
                n_acc = 0
                for k in range(KCH):
                    n_acc += 2  # uT.aT + sT.aT
                    b_lo, b_hi = t0 // P, (t0 + width) // P
                    for b in range(b_lo, b_hi):
                        n_acc += 1  # uT.rT per block
                i_acc = 0

                def acc(stat, mov_ap, col0, ncols):
                    nonlocal i_acc
                    nc.tensor.matmul(
                        lt[:, col0 : col0 + ncols],
                        stat,
                        mov_ap,
                        start=(i_acc == 0),
                        stop=(i_acc == n_acc - 1),
                    )
                    i_acc += 1

                for k in range(KCH):
                    acc(uT[:, k, :], at[:, k, t0 : t0 + width], 0, width)
                    for b in range(t0 // P, (t0 + width) // P):
                        col0 = b * P - t0
                        if b == 0:
                            acc(uT[:, k, :], rt1[:, k, :], col0, P)
                        else:
                            acc(uT[:, k, :], rt2[:, k, (b - 1) * P : b * P], col0, P)
                    acc(sT[:, k, :], at[:, k, t0 : t0 + width], 0, width)
                return lt

            def compute_post(m, lt, t0=0, width=MEGA):
                nb = width // P  # token blocks in this slice
                b0 = t0 // P
                lts = ltspool.tile([E, width], f32, tag="lts")
                nc.scalar.copy(lts[:], lt[:])

                # transpose logits back -> [128t, 64e] blocks in PSUM (fp32)
                lg = pslg.tile([P, nb * E], f32, tag="lg")
                for b in range(nb):
                    nc.tensor.transpose(
                        lg[:, b * E : (b + 1) * E],
                        lts[:, b * P : (b + 1) * P],
                        idf[0:E, 0:E],
                    )

                # --- top-k pipeline, all nb token-blocks fused per op ---
                lg3 = lg[:].rearrange("p (b e) -> p b e", b=nb)
                # e = exp(logit - max): keeps ACT exp args in [-24, 0] where
                # the table is ~4x more accurate (fewer selection-flip risks
                # near group-boundary ties). Per-block bias via DVE subtract.
                nmax = spool.tile([P, nb], f32, tag="nmax")
                nc.vector.tensor_reduce(
                    nmax[:], lg3, axis=X, op=mybir.AluOpType.max, negate=True
                )
                e_sb = spool.tile([P, nb, E], f32, tag="esb")
                for b in range(nb):
                    nc.scalar.activation(
                        e_sb[:, b, :],
                        lg3[:, b, :],
                        mybir.ActivationFunctionType.Exp,
                        bias=nmax[:, b : b + 1],
                    )
                e4 = e_sb[:].rearrange("p b (g j) -> p b g j", g=G)
                gmax = spool.tile([P, nb, G], f32, tag="gmax")
                nc.vector.tensor_reduce(
                    gmax[:], e4, axis=X, op=mybir.AluOpType.max
                )
                gsort = spool.tile([P, nb, 8], f32, tag="gsort")
                for b in range(nb):
                    nc.vector.max(gsort[:, b, :], gmax[:, b, :])
                gmask = spool.tile([P, nb, G], f32, tag="gmask")
                nc.vector.tensor_tensor(
                    gmask[:],
                    gmax[:],
                    gsort[:, :, TG - 1 : TG].broadcast_to([P, nb, G]),
                    mybir.AluOpType.is_ge,
                )
                me = spool.tile([P, nb, E], f32, tag="me")
                nc.vector.tensor_tensor(
                    me[:].rearrange("p b (g j) -> p b g j", g=G),
                    e4,
                    gmask[:].unsqueeze(3).broadcast_to([P, nb, G, PG]),
                    mybir.AluOpType.mult,
                )
                t8 = spool.tile([P, nb, 8], f32, tag="t8")
                for b in range(nb):
                    nc.vector.max(t8[:, b, :], me[:, b, :])
                ssum = spool.tile([P, nb], f32, tag="ssum")
                nc.vector.tensor_reduce(
                    ssum[:], t8[:, :, 0:TK], axis=X, op=mybir.AluOpType.add
                )
                rec = spool.tile([P, nb], f32, tag="rec")
                nc.vector.reciprocal(rec[:], ssum[:])
                ow = opool.tile([P, nb, TK], f32, tag="ow")
                nc.vector.tensor_tensor(
                    ow[:],
                    t8[:, :, 0:TK],
                    rec[:].unsqueeze(2).broadcast_to([P, nb, TK]),
                    mybir.AluOpType.mult,
                )
                nc.gpsimd.dma_start(out=our[:, m, b0 : b0 + nb], in_=ow[:])

            # software pipeline:
            #   fronts(m) -> matmuls(m-1) -> backs(m) -> post(m-1)
            prev = None  # (m, lt PSUM tile)
            prev_t = None  # (at, rt) awaiting matmul
            w_done = False
            warm_pe()
            for _r in range(repeat):
                for m in range(NM):
                    quarters = front_megatile(m)
                    if not w_done:
                        setup_w()
                        w_done = True
                    if prev_t is not None:
                        lt = compute_mm(*prev_t[1:])
                        prev = (prev_t[0], lt)
                    cur_t = (m, *back_megatile(m, quarters))
                    if prev is not None:
                        compute_post(*prev)
                        prev = None
                    prev_t = cur_t
            if prev_t is not None:
                # split the final megatile so its top-k overlaps the second
                # half-chain instead of serializing after the last matmul
                m_l, at_l, rt_l = prev_t
                lt1 = compute_mm(at_l, rt_l, 0, MEGA // 2)
                compute_post(m_l, lt1, 0, MEGA // 2)
                lt2 = compute_mm(at_l, rt_l, MEGA // 2, MEGA // 2)
                compute_post(m_l, lt2, MEGA // 2, MEGA // 2)

    nc.compile()
    return nc


_NC_CACHE = {}


def _get_nc(t_core):
    if t_core not in _NC_CACHE:
        _NC_CACHE[t_core] = build_nc(t_core)
    return _NC_CACHE[t_core]


def run_sharded(flat_x, w, trace=False, **kw):
    """flat_x: [T, H] f32. Returns ([T, 6] f32, BassKernelResults)."""
    from concourse.bass_utils import run_bass_kernel_spmd

    T = flat_x.shape[0]
    tc = T // N_CORES
    nc = _get_nc(tc)
    in_maps = [
        {"x": np.ascontiguousarray(flat_x[i * tc : (i + 1) * tc]), "w": w}
        for i in range(N_CORES)
    ]
    res = run_bass_kernel_spmd(nc, in_maps, list(range(N_CORES)), trace=trace, **kw)
    outs = [np.asarray(res.results[i]["out"]) for i in range(N_CORES)]
    return np.concatenate(outs, axis=0), res


def kernel(hidden_states, kernel):
    hs = np.asarray(hidden_states, dtype=np.float32)
    w = np.ascontiguousarray(np.asarray(kernel, dtype=np.float32))
    B, S, Hh = hs.shape
    flat = np.ascontiguousarray(hs.reshape(B * S, Hh))
    out, _ = run_sharded(flat, w)
    return out


# revision 11
# speedup vs baseline: 1.1772x; 1.0511x over previous
"""MoE gate (DeepSeek-V2 style, group-limited greedy top-k) for Trainium2.

Full-input contract: kernel(hidden_states[4,8192,2048] f32, kernel[64,2048] f32)
-> topk_weight [32768, 6] f32.

Strategy: pure data-parallel over 8 NeuronCores (4096 tokens each).

Numerics: x is decomposed on-chip as x = a + r with a = f16(x) and
r = f16(x - a) (engine copy + subtract; the rounding mode of a is irrelevant
because r compensates exactly); w is decomposed once as w = u + s the same
way. Then three f16 matmul passes

    logitsT += uT.aT + uT.rT + sT.aT        (skipped term rT.sT ~ 2^-22)

reproduce fp32 logits to ~4e-6 absolute (measured on HW), far below the
~1e-5 group-selection margin of this input, while running at 1 cycle/row on
the PE -- and, unlike the f32r hi/lo scheme, the operands are 2-byte so the
transposition moves off the PE entirely:

Transposition: every [128t x 128h] chunk of a and r is transposed
SBUF->SBUF by the XBAR DMA-transpose engine on the SP/ACT HWDGE queues
(~56 ns of queue time per chunk in the cost model), eliminating both the PE
transpose passes and the PSUM->SBUF evictions of the f32r design.
(The gpsimd cast-DMA (f32->f16 straight from DRAM) would be cheaper still,
but on real HW an XBAR transpose reading an SBUF region written by a SWDGE
DMA races with it -- measured data corruption -- so a is engine-computed.)

Per core: tokens are remapped so partition p owns a contiguous 32-token DRAM
range (t = p*32 + m*4 + b), making every DMA descriptor large & contiguous.
Issue order is software-pipelined at megatile granularity:
fronts(m) [loads + a/r ops] -> matmuls(m-1) -> backs(m) [transposes] ->
post(m-1) [lts, logits back-transpose, top-k, store], so no engine FIFO
blocks on a dependency issued just before it.

Top-k per 128-token block on DVE/Pool/ACT using the hardware top-8 sort
(InstMax): softmax denominator cancels in the final normalization, so only
e = exp(logit - max) is needed; group-max -> sort -> 3rd value threshold ->
group mask -> masked e -> top-8 sort -> sum top-6 -> reciprocal -> scale.

Engine budget per core (cost model, 4096 tokens):
  PE   48 f16 mm/megatile + lg transposes + warmup ~87 us   <- critical
  SP   16 f32 loads + half the transposes + stores ~82 us
  ACT  24 a-ops + half transposes + exp/lts        ~84 us
  Pool 16 f32 loads + 4 a + 4 r + lsub/me          ~87 us
  DVE  28 r + 4 a + rest of top-k                  ~85 us
"""

import sys

if "/opt/trn_rl_repo" not in sys.path:
    sys.path.insert(0, "/opt/trn_rl_repo")

import numpy as np

# Problem constants (hardcoded per contract)
N_CORES = 8
H = 2048
E = 64  # n_routed_experts
G = 8  # n_group
PG = E // G  # experts per group
TG = 3  # topk_group
TK = 6  # top_k
P = 128  # partitions
MEGA = 512  # tokens per megatile
BB = MEGA // P  # 4 token blocks per megatile
KCH = H // P  # 16 contraction chunks


def build_nc(t_core, repeat=1):
    """Build the single-core Bass program for a t_core-token shard."""
    from concourse import bacc, mybir, masks
    from concourse.tile import TileContext

    f32 = mybir.dt.float32
    f16 = mybir.dt.float16
    X = mybir.AxisListType.X
    NM = t_core // MEGA
    assert t_core % MEGA == 0

    nc = bacc.Bacc()
    x = nc.declare_dram_parameter("x", [t_core, H], f32, isOutput=False)
    w = nc.declare_dram_parameter("w", [E, H], f32, isOutput=False)
    out = nc.declare_dram_parameter("out", [t_core, TK], f32, isOutput=True)

    with TileContext(nc) as tc:
        with (
            tc.tile_pool(name="const", bufs=1) as cpool,
            tc.tile_pool(name="xq", bufs=5) as xqpool,
            tc.tile_pool(name="aq", bufs=6) as aqpool,
            tc.tile_pool(name="at", bufs=2) as atpool,
            tc.tile_pool(name="rt", bufs=2) as rtpool,
            tc.tile_pool(name="lts", bufs=2) as ltspool,
            tc.tile_pool(name="small", bufs=2) as spool,
            tc.tile_pool(name="outp", bufs=2) as opool,
            tc.tile_pool(name="ps_mm", bufs=2, space="PSUM") as psmm,
            tc.tile_pool(name="ps_lg", bufs=2, space="PSUM") as pslg,
            tc.tile_pool(name="ps_wm", bufs=1, space="PSUM") as pswm,
        ):
            identf = cpool.tile([P, P], f32)
            masks.make_identity(nc, identf[:])
            idf = identf[:]

            w_sb = cpool.tile([E, H], f32)
            u_sb = cpool.tile([E, H], f16)
            s_sb = cpool.tile([E, H], f16)
            uT = cpool.tile([P, KCH, E], f16)
            sT = cpool.tile([P, KCH, E], f16)

            def warm_pe(n=26):
                # Dummy identity transposes burn through the PE p-state ramp
                # (~3us of continuous activity) during the otherwise PE-idle
                # DMA head, so real matmuls start at full clock.
                pwm = pswm.tile([P, P], f32, tag="wm")
                for _ in range(n):
                    nc.tensor.transpose(pwm[:], idf, idf)

            def setup_w():
                # w -> u = f16(w), s = f16(w - u); DMA-transpose both to
                # [128h, k, 64e]. Issued after megatile 0's fronts so it
                # doesn't gate the pipeline head.
                nc.gpsimd.dma_start(out=w_sb[:], in_=w[:])
                nc.scalar.copy(u_sb[:], w_sb[:])
                nc.vector.tensor_tensor(
                    s_sb[:], w_sb[:], u_sb[:], mybir.AluOpType.subtract
                )
                for k in range(KCH):
                    nc.scalar.dma_start(
                        out=uT[:, k, :],
                        in_=u_sb[:, k * P : (k + 1) * P],
                        transpose=True,
                    )
                    nc.scalar.dma_start(
                        out=sT[:, k, :],
                        in_=s_sb[:, k * P : (k + 1) * P],
                        transpose=True,
                    )

            xr = x[:].rearrange("(p m b) h -> p m b h", p=P, m=NM, b=BB)
            our = out[:].rearrange("(p m b) k -> p m b k", p=P, m=NM, b=BB)

            def front_megatile(m):
                # Per quarter b: f32 load (SP / Pool copy rings), a = f16(x)
                # (DVE / Pool), r = f16(x - a) (DVE), into one combined arq
                # tile. The ACT HWDGE ring carries ONLY XBAR transposes:
                # concurrent transposes on two rings -- or transposes mixed
                # with copies in one ring -- corrupt data on real HW.
                quarters = []
                for b in range(BB):
                    xt = xqpool.tile([P, H], f32, tag="xq")
                    load_eng = nc.sync if b < 2 or (b == 2 and m % 8 < 1) else nc.gpsimd
                    load_eng.dma_start(out=xt[:], in_=xr[:, m, b, :])
                    arq = aqpool.tile([P, 2, H], f16, tag="arq")
                    if b < 2 or (b == 2 and m % 2 == 0):
                        nc.gpsimd.tensor_copy(arq[:, 0, :], xt[:])
                    else:
                        nc.vector.tensor_copy(arq[:, 0, :], xt[:])
                    nc.vector.tensor_tensor(
                        arq[:, 1, :], xt[:], arq[:, 0, :],
                        mybir.AluOpType.subtract,
                    )
                    quarters.append(arq)
                return quarters

            def back_megatile(m, quarters):
                # 64 XBAR transposes per quarter-pair into aT/rT, all on the
                # ACT HWDGE ring (the only ring carrying transposes). A tiny
                # 16-row dummy transpose fences each quarter: it waits on the
                # a/r producer semaphores and its pipeline latency lets the
                # engine writes settle in SBUF before the real XBAR reads.
                at = atpool.tile([P, KCH, MEGA], f16, tag="at")
                rt = rtpool.tile([P, KCH, MEGA], f16, tag="rt")
                for b, arq in enumerate(quarters):
                    fd = spool.tile([P, P], f16, tag="fd")
                    nc.scalar.dma_start(
                        out=fd[0:P, 0:16], in_=arq[0:16, 1, 0:P], transpose=True
                    )
                    for k in range(KCH):
                        nc.scalar.dma_start(
                            out=at[:, k, b * P : (b + 1) * P],
                            in_=arq[:, 0, k * P : (k + 1) * P],
                            transpose=True,
                        )
                        nc.scalar.dma_start(
                            out=rt[:, k, b * P : (b + 1) * P],
                            in_=arq[:, 1, k * P : (k + 1) * P],
                            transpose=True,
                        )
                return at, rt

            def compute_mm(at, rt, t0=0, width=MEGA):
                # logitsT[64, width] += uT.aT + uT.rT + sT.aT (f16, 1 cyc/row)
                lt = psmm.tile([E, width], f32, tag="lt")
                n_acc = 3 * KCH
                i_acc = 0
                for k in range(KCH):
                    for wt_k, xt_k in ((uT, at), (uT, rt), (sT, at)):
                        nc.tensor.matmul(
                            lt[:],
                            wt_k[:, k, :],
                            xt_k[:, k, t0 : t0 + width],
                            start=(i_acc == 0),
                            stop=(i_acc == n_acc - 1),
                        )
                        i_acc += 1
                return lt

            def compute_post(m, lt, t0=0, width=MEGA):
                nb = width // P  # token blocks in this slice
                b0 = t0 // P
                lts = ltspool.tile([E, width], f32, tag="lts")
                nc.vector.tensor_copy(lts[:], lt[:])

                # transpose logits back -> [128t, 64e] blocks in PSUM (fp32)
                lg = pslg.tile([P, nb * E], f32, tag="lg")
                for b in range(nb):
                    nc.tensor.transpose(
                        lg[:, b * E : (b + 1) * E],
                        lts[:, b * P : (b + 1) * P],
                        idf[0:E, 0:E],
                    )

                # --- top-k pipeline, all nb token-blocks fused per op ---
                lg3 = lg[:].rearrange("p (b e) -> p b e", b=nb)
                # e = exp(logit - max): keeps ACT exp args in [-24, 0] where
                # the table is ~4x more accurate (fewer selection-flip risks
                # near group-boundary ties). Per-block bias via DVE subtract.
                nmax = spool.tile([P, nb], f32, tag="nmax")
                nc.vector.tensor_reduce(
                    nmax[:], lg3, axis=X, op=mybir.AluOpType.max, negate=True
                )
                e_sb = spool.tile([P, nb, E], f32, tag="esb")
                for b in range(nb):
                    nc.scalar.activation(
                        e_sb[:, b, :],
                        lg3[:, b, :],
                        mybir.ActivationFunctionType.Exp,
                        bias=nmax[:, b : b + 1],
                    )
                e4 = e_sb[:].rearrange("p b (g j) -> p b g j", g=G)
                gmax = spool.tile([P, nb, G], f32, tag="gmax")
                nc.vector.tensor_reduce(
                    gmax[:], e4, axis=X, op=mybir.AluOpType.max
                )
                gsort = spool.tile([P, nb, 8], f32, tag="gsort")
                for b in range(nb):
                    nc.vector.max(gsort[:, b, :], gmax[:, b, :])
                gmask = spool.tile([P, nb, G], f32, tag="gmask")
                nc.vector.tensor_tensor(
                    gmask[:],
                    gmax[:],
                    gsort[:, :, TG - 1 : TG].broadcast_to([P, nb, G]),
                    mybir.AluOpType.is_ge,
                )
                me = spool.tile([P, nb, E], f32, tag="me")
                nc.vector.tensor_tensor(
                    me[:].rearrange("p b (g j) -> p b g j", g=G),
                    e4,
                    gmask[:].unsqueeze(3).broadcast_to([P, nb, G, PG]),
                    mybir.AluOpType.mult,
                )
                t8 = spool.tile([P, nb, 8], f32, tag="t8")
                for b in range(nb):
                    nc.vector.max(t8[:, b, :], me[:, b, :])
                ssum = spool.tile([P, nb], f32, tag="ssum")
                nc.vector.tensor_reduce(
                    ssum[:], t8[:, :, 0:TK], axis=X, op=mybir.AluOpType.add
                )
                rec = spool.tile([P, nb], f32, tag="rec")
                nc.vector.reciprocal(rec[:], ssum[:])
                ow = opool.tile([P, nb, TK], f32, tag="ow")
                nc.vector.tensor_tensor(
                    ow[:],
                    t8[:, :, 0:TK],
                    rec[:].unsqueeze(2).broadcast_to([P, nb, TK]),
                    mybir.AluOpType.mult,
                )
                nc.gpsimd.dma_start(out=our[:, m, b0 : b0 + nb], in_=ow[:])

            # software pipeline:
            #   fronts(m) -> matmuls(m-1) -> backs(m) -> post(m-1)
            prev = None  # (m, lt PSUM tile)
            prev_t = None  # (at, rt) awaiting matmul
            w_done = False
            warm_pe()
            for _r in range(repeat):
                for m in range(NM):
                    quarters = front_megatile(m)
                    if not w_done:
                        setup_w()
                        w_done = True
                    if prev_t is not None:
                        lt = compute_mm(*prev_t[1:])
                        prev = (prev_t[0], lt)
                    cur_t = (m, *back_megatile(m, quarters))
                    if prev is not None:
                        compute_post(*prev)
                        prev = None
                    prev_t = cur_t
            if prev_t is not None:
                # split the final megatile so its top-k overlaps the second
                # half-chain instead of serializing after the last matmul
                m_l, at_l, rt_l = prev_t
                lt1 = compute_mm(at_l, rt_l, 0, MEGA // 2)
                compute_post(m_l, lt1, 0, MEGA // 2)
                lt2 = compute_mm(at_l, rt_l, MEGA // 2, MEGA // 2)
                compute_post(m_l, lt2, MEGA // 2, MEGA // 2)

    nc.compile()
    return nc


_NC_CACHE = {}


def _get_nc(t_core):
    if t_core not in _NC_CACHE:
        _NC_CACHE[t_core] = build_nc(t_core)
    return _NC_CACHE[t_core]


def run_sharded(flat_x, w, trace=False, **kw):
    """flat_x: [T, H] f32. Returns ([T, 6] f32, BassKernelResults)."""
    from concourse.bass_utils import run_bass_kernel_spmd

    T = flat_x.shape[0]
    tc = T // N_CORES
    nc = _get_nc(tc)
    in_maps = [
        {"x": np.ascontiguousarray(flat_x[i * tc : (i + 1) * tc]), "w": w}
        for i in range(N_CORES)
    ]
    res = run_bass_kernel_spmd(nc, in_maps, list(range(N_CORES)), trace=trace, **kw)
    outs = [np.asarray(res.results[i]["out"]) for i in range(N_CORES)]
    return np.concatenate(outs, axis=0), res


def kernel(hidden_states, kernel):
    hs = np.asarray(hidden_states, dtype=np.float32)
    w = np.ascontiguousarray(np.asarray(kernel, dtype=np.float32))
    B, S, Hh = hs.shape
    flat = np.ascontiguousarray(hs.reshape(B * S, Hh))
    out, _ = run_sharded(flat, w)
    return out


# revision 15
# speedup vs baseline: 2.2339x; 1.8976x over previous
"""MoE gate (DeepSeek-V2 style, group-limited greedy top-k) for Trainium2.

Full-input contract: kernel(hidden_states[4,8192,2048] f32, kernel[64,2048] f32)
-> topk_weight [32768, 6] f32.

Strategy: pure data-parallel over 8 NeuronCores (4096 tokens each).
Per core:
  - tokens are remapped so partition p owns a contiguous 32-token DRAM range
    (t = p*32 + m*4 + b), making every DMA descriptor large & contiguous.
  - per 512-token megatile: DMA x -> SBUF [128, 4, 2048]; PE-transpose
    (float32r mode, exact fp32 bits) into PSUM; copy PSUM->SBUF xT
    [128h, 512t] alternating ACT/DVE engines; accumulate logitsT[64, 512]
    over 16 h-chunks with float32r matmuls (W stationary); PE-transpose
    logits back to [128t, 64e]; then a per-128-token top-k pipeline on
    DVE/ACT using the hardware top-8 sort (InstMax):
      softmax denominator cancels in the final normalization, so we only
      need e = exp(logit - max); group-max -> sort -> 3rd value threshold
      -> group mask -> masked e -> top-8 sort -> sum top-6 -> reciprocal
      -> scale.
"""

import sys

if "/opt/trn_rl_repo" not in sys.path:
    sys.path.insert(0, "/opt/trn_rl_repo")

import numpy as np

# Problem constants (hardcoded per contract)
N_CORES = 8
H = 2048
E = 64  # n_routed_experts
G = 8  # n_group
PG = E // G  # experts per group
TG = 3  # topk_group
TK = 6  # top_k
P = 128  # partitions
MEGA = 512  # tokens per megatile
BB = MEGA // P  # 4 token blocks per megatile
KCH = H // P  # 16 contraction chunks


def build_nc(t_core, repeat=1):
    """Build the single-core Bass program for a t_core-token shard.

    repeat>1 re-runs the whole pipeline (timing experiments only).
    """
    from concourse import bacc, mybir, masks
    from concourse.tile import TileContext

    f32 = mybir.dt.float32
    f32r = mybir.dt.float32r
    X = mybir.AxisListType.X
    NM = t_core // MEGA
    assert t_core % MEGA == 0

    nc = bacc.Bacc()
    x = nc.declare_dram_parameter("x", [t_core, H], f32, isOutput=False)
    w = nc.declare_dram_parameter("w", [E, H], f32, isOutput=False)
    out = nc.declare_dram_parameter("out", [t_core, TK], f32, isOutput=True)

    with TileContext(nc) as tc:
        with (
            tc.tile_pool(name="const", bufs=1) as cpool,
            tc.tile_pool(name="xin", bufs=6) as xpool,
            tc.tile_pool(name="xhi", bufs=2) as xhipool,
            tc.tile_pool(name="xlo", bufs=2) as xlopool,
            tc.tile_pool(name="lts", bufs=2) as ltspool,
            tc.tile_pool(name="small", bufs=2) as spool,
            tc.tile_pool(name="outp", bufs=2) as opool,
            tc.tile_pool(name="ps_t", bufs=5, space="PSUM") as pst,
            tc.tile_pool(name="ps_mm", bufs=2, space="PSUM") as psmm,
            tc.tile_pool(name="ps_lg", bufs=1, space="PSUM") as pslg,
        ):
            identf = cpool.tile([P, P], f32)
            masks.make_identity(nc, identf[:])
            idf = identf[:]

            w_sb = cpool.tile([E, H], f32)
            w_hi = cpool.tile([P, KCH, E], f32r)
            w_lo = cpool.tile([P, KCH, E], f32r)

            def warm_pe(n=24):
                # Dummy identity transposes fill the otherwise-idle DMA head
                # and burn through the PE p-state ramp (P3/HAM warmup), so
                # real transposes start at full clock.
                pwm = pslg.tile([P, P], f32, tag="lg")
                for _ in range(n):
                    nc.tensor.transpose(pwm[:], idf, idf)

            def setup_w():
                # W: load + transpose once -> w_hi/w_lo [128h, k, 64e] f32r
                # (hi/lo split so that 3 f32r matmuls reach fp32 accuracy).
                # Issued after megatile 0's loads so it doesn't gate the head;
                # chunked so the first W transposes start early.
                nc.scalar.dma_start(out=w_sb[:], in_=w[:])
                for k in range(KCH):
                    pw = psmm.tile([P, E], f32, tag="lt")
                    nc.tensor.transpose(
                        pw[:, 0:E],
                        w_sb[:, k * P : (k + 1) * P],
                        idf[0:E, 0:E],
                    )
                    nc.vector.tensor_copy(w_hi[:, k, :], pw[:, 0:E])
                    nc.vector.tensor_tensor(
                        w_lo[:, k, :], pw[:, 0:E], w_hi[:, k, :],
                        mybir.AluOpType.subtract,
                    )

            xr = x[:].rearrange("(p m b) h -> p m b h", p=P, m=NM, b=BB)
            our = out[:].rearrange("(p m b) k -> p m b k", p=P, m=NM, b=BB)

            def load_and_transpose(m, hsplit=False):
                # Loads alternate the two HWDGE rings (SP + ACT). Steady
                # state: one load per token-quarter. Megatile 0 (hsplit):
                # split along H instead, so transpose chunk k waits only on
                # h-quarter k//4 and the pipeline fills ~3us earlier.
                xq = []
                HQ = H // BB
                for c in range(BB):
                    eng = nc.sync if c % 2 == 0 else nc.scalar
                    if hsplit:
                        t = xpool.tile([P, BB, HQ], f32, tag="xin")
                        eng.dma_start(
                            out=t[:], in_=xr[:, m, :, c * HQ : (c + 1) * HQ]
                        )
                    else:
                        t = xpool.tile([P, H], f32, tag="xin")
                        eng.dma_start(out=t[:], in_=xr[:, m, c, :])
                    xq.append(t)

                def src(k, b):
                    if hsplit:
                        kq = HQ // P
                        return xq[k // kq][:, b, (k % kq) * P : (k % kq + 1) * P]
                    return xq[b][:, k * P : (k + 1) * P]

                x_hi = xhipool.tile([P, KCH, MEGA], f32r)
                x_lo = xlopool.tile([P, KCH, MEGA], f32r)
                for k in range(KCH):
                    pt = pst.tile([P, MEGA], f32, tag="pt")
                    for b in range(BB):
                        nc.tensor.transpose(
                            pt[:, b * P : (b + 1) * P],
                            src(k, b),
                            idf,
                        )
                    # hi = f32r(x) on ACT (1-input); lo = f32r(x - hi) on DVE
                    nc.scalar.copy(x_hi[:, k, :], pt[:])
                    nc.vector.tensor_tensor(
                        x_lo[:, k, :], pt[:], x_hi[:, k, :],
                        mybir.AluOpType.subtract,
                    )
                return x_hi, x_lo

            def compute(m, x_hi, x_lo, t0=0, width=MEGA):
                nb = width // P  # token blocks in this slice
                b0 = t0 // P
                # logitsT[64, width] += w_hi.x_hi + w_hi.x_lo + w_lo.x_hi
                lt = psmm.tile([E, width], f32, tag="lt")
                n_acc = 3 * KCH
                i_acc = 0
                for k in range(KCH):
                    for wt_k, xt_k in (
                        (w_hi, x_hi),
                        (w_hi, x_lo),
                        (w_lo, x_hi),
                    ):
                        nc.tensor.matmul(
                            lt[:],
                            wt_k[:, k, :],
                            xt_k[:, k, t0 : t0 + width],
                            start=(i_acc == 0),
                            stop=(i_acc == n_acc - 1),
                        )
                        i_acc += 1
                lts = ltspool.tile([E, width], f32, tag="lts")
                nc.scalar.copy(lts[:], lt[:])

                # transpose logits back -> [128t, 64e] blocks in PSUM (fp32)
                lg = pslg.tile([P, nb * E], f32, tag="lg")
                for b in range(nb):
                    nc.tensor.transpose(
                        lg[:, b * E : (b + 1) * E],
                        lts[:, b * P : (b + 1) * P],
                        idf[0:E, 0:E],
                    )

                # --- top-k pipeline, all nb token-blocks fused per op ---
                BB = nb
                lg3 = lg[:].rearrange("p (b e) -> p b e", b=BB)  # [128,nb,64]
                # e = exp(logit - max): keeps ACT exp args in [-24, 0] where
                # the table is ~4x more accurate (fewer selection-flip risks
                # near group-boundary ties). Per-block bias via DVE subtract.
                nmax = spool.tile([P, BB], f32, tag="nmax")
                nc.vector.tensor_reduce(
                    nmax[:], lg3, axis=X, op=mybir.AluOpType.max, negate=True
                )
                lsub = spool.tile([P, BB, E], f32, tag="lsub")
                nc.vector.tensor_tensor(
                    lsub[:],
                    lg3,
                    nmax[:].unsqueeze(2).broadcast_to([P, BB, E]),
                    mybir.AluOpType.add,
                )
                e_sb = spool.tile([P, BB, E], f32, tag="esb")
                nc.scalar.activation(
                    e_sb[:], lsub[:], mybir.ActivationFunctionType.Exp
                )
                e4 = e_sb[:].rearrange("p b (g j) -> p b g j", g=G)
                gmax = spool.tile([P, BB, G], f32, tag="gmax")
                nc.vector.tensor_reduce(
                    gmax[:], e4, axis=X, op=mybir.AluOpType.max
                )
                gsort = spool.tile([P, BB, 8], f32, tag="gsort")
                for b in range(BB):
                    nc.vector.max(gsort[:, b, :], gmax[:, b, :])
                gmask = spool.tile([P, BB, G], f32, tag="gmask")
                nc.vector.tensor_tensor(
                    gmask[:],
                    gmax[:],
                    gsort[:, :, TG - 1 : TG].broadcast_to([P, BB, G]),
                    mybir.AluOpType.is_ge,
                )
                me = spool.tile([P, BB, E], f32, tag="me")
                nc.vector.tensor_tensor(
                    me[:].rearrange("p b (g j) -> p b g j", g=G),
                    e4,
                    gmask[:].unsqueeze(3).broadcast_to([P, BB, G, PG]),
                    mybir.AluOpType.mult,
                )
                t8 = spool.tile([P, BB, 8], f32, tag="t8")
                for b in range(BB):
                    nc.vector.max(t8[:, b, :], me[:, b, :])
                ssum = spool.tile([P, BB], f32, tag="ssum")
                nc.vector.tensor_reduce(
                    ssum[:], t8[:, :, 0:TK], axis=X, op=mybir.AluOpType.add
                )
                rec = spool.tile([P, BB], f32, tag="rec")
                nc.vector.reciprocal(rec[:], ssum[:])
                ow = opool.tile([P, BB, TK], f32, tag="ow")
                nc.vector.tensor_tensor(
                    ow[:],
                    t8[:, :, 0:TK],
                    rec[:].unsqueeze(2).broadcast_to([P, BB, TK]),
                    mybir.AluOpType.mult,
                )
                nc.sync.dma_start(out=our[:, m, b0 : b0 + nb], in_=ow[:])

            # two-stage software pipeline: transposes/copies of megatile m
            # are issued alongside the matmuls/topk of megatile m-1 so the
            # PE never waits on PSUM->SBUF copies of the tile it multiplies.
            prev = None
            w_done = False
            warm_pe()
            for _r in range(repeat):
                for m in range(NM):
                    cur = (m, *load_and_transpose(m))
                    if not w_done:
                        setup_w()
                        w_done = True
                    if prev is not None:
                        compute(*prev)
                    prev = cur
            if prev is not None:
                # split the final megatile so its top-k overlaps the second
                # half-chain instead of serializing after the last matmul
                m_l, xh_l, xl_l = prev
                compute(m_l, xh_l, xl_l, 0, MEGA // 2)
                compute(m_l, xh_l, xl_l, MEGA // 2, MEGA // 2)

    nc.compile()
    return nc


_NC_CACHE = {}


def _get_nc(t_core):
    if t_core not in _NC_CACHE:
        _NC_CACHE[t_core] = build_nc(t_core)
    return _NC_CACHE[t_core]


def run_sharded(flat_x, w, trace=False, **kw):
    """flat_x: [T, H] f32. Returns ([T, 6] f32, BassKernelResults)."""
    from concourse.bass_utils import run_bass_kernel_spmd

    T = flat_x.shape[0]
    tc = T // N_CORES
    nc = _get_nc(tc)
    in_maps = [
        {"x": np.ascontiguousarray(flat_x[i * tc : (i + 1) * tc]), "w": w}
        for i in range(N_CORES)
    ]
    res = run_bass_kernel_spmd(nc, in_maps, list(range(N_CORES)), trace=trace, **kw)
    outs = [np.asarray(res.results[i]["out"]) for i in range(N_CORES)]
    return np.concatenate(outs, axis=0), res


def kernel(hidden_states, kernel):
    hs = np.asarray(hidden_states, dtype=np.float32)
    w = np.ascontiguousarray(np.asarray(kernel, dtype=np.float32))
    B, S, Hh = hs.shape
    flat = np.ascontiguousarray(hs.reshape(B * S, Hh))
    out, _ = run_sharded(flat, w)
    return out


# revision 22
# speedup vs baseline: 2.2340x; 1.0000x over previous
"""MoE gate (DeepSeek-V2 style, group-limited greedy top-k) for Trainium2.

Full-input contract: kernel(hidden_states[4,8192,2048] f32, kernel[64,2048] f32)
-> topk_weight [32768, 6] f32.

Strategy: pure data-parallel over 8 NeuronCores (4096 tokens each).
Per core:
  - tokens are remapped so partition p owns a contiguous 32-token DRAM range
    (t = p*32 + m*4 + b), making every DMA descriptor large & contiguous.
  - per 512-token megatile: DMA x -> SBUF [128, 4, 2048]; PE-transpose
    (float32r mode, exact fp32 bits) into PSUM; copy PSUM->SBUF xT
    [128h, 512t] alternating ACT/DVE engines; accumulate logitsT[64, 512]
    over 16 h-chunks with float32r matmuls (W stationary); PE-transpose
    logits back to [128t, 64e]; then a per-128-token top-k pipeline on
    DVE/ACT using the hardware top-8 sort (InstMax):
      softmax denominator cancels in the final normalization, so we only
      need e = exp(logit - max); group-max -> sort -> 3rd value threshold
      -> group mask -> masked e -> top-8 sort -> sum top-6 -> reciprocal
      -> scale.
"""

import sys

if "/opt/trn_rl_repo" not in sys.path:
    sys.path.insert(0, "/opt/trn_rl_repo")

import numpy as np

# Problem constants (hardcoded per contract)
N_CORES = 8
H = 2048
E = 64  # n_routed_experts
G = 8  # n_group
PG = E // G  # experts per group
TG = 3  # topk_group
TK = 6  # top_k
P = 128  # partitions
MEGA = 512  # tokens per megatile
BB = MEGA // P  # 4 token blocks per megatile
KCH = H // P  # 16 contraction chunks


def build_nc(t_core, repeat=1):
    """Build the single-core Bass program for a t_core-token shard.

    repeat>1 re-runs the whole pipeline (timing experiments only).
    """
    from concourse import bacc, mybir, masks
    from concourse.tile import TileContext

    f32 = mybir.dt.float32
    f32r = mybir.dt.float32r
    X = mybir.AxisListType.X
    NM = t_core // MEGA
    assert t_core % MEGA == 0

    nc = bacc.Bacc()
    x = nc.declare_dram_parameter("x", [t_core, H], f32, isOutput=False)
    w = nc.declare_dram_parameter("w", [E, H], f32, isOutput=False)
    out = nc.declare_dram_parameter("out", [t_core, TK], f32, isOutput=True)

    with TileContext(nc) as tc:
        with (
            tc.tile_pool(name="const", bufs=1) as cpool,
            tc.tile_pool(name="xin", bufs=6) as xpool,
            tc.tile_pool(name="xhi", bufs=2) as xhipool,
            tc.tile_pool(name="xlo", bufs=2) as xlopool,
            tc.tile_pool(name="lts", bufs=2) as ltspool,
            tc.tile_pool(name="small", bufs=2) as spool,
            tc.tile_pool(name="outp", bufs=2) as opool,
            tc.tile_pool(name="ps_t", bufs=5, space="PSUM") as pst,
            tc.tile_pool(name="ps_mm", bufs=2, space="PSUM") as psmm,
            tc.tile_pool(name="ps_lg", bufs=1, space="PSUM") as pslg,
        ):
            identf = cpool.tile([P, P], f32)
            masks.make_identity(nc, identf[:])
            idf = identf[:]

            w_sb = cpool.tile([E, H], f32)
            w_hi = cpool.tile([P, KCH, E], f32r)
            w_lo = cpool.tile([P, KCH, E], f32r)

            def warm_pe(n=24):
                # Dummy identity transposes fill the otherwise-idle DMA head
                # and burn through the PE p-state ramp (P3/HAM warmup), so
                # real transposes start at full clock.
                pwm = pslg.tile([P, P], f32, tag="lg")
                for _ in range(n):
                    nc.tensor.transpose(pwm[:], idf, idf)

            def setup_w():
                # W: load + transpose once -> w_hi/w_lo [128h, k, 64e] f32r
                # (hi/lo split so that 3 f32r matmuls reach fp32 accuracy).
                # Issued after megatile 0's loads so it doesn't gate the head;
                # chunked so the first W transposes start early.
                nc.scalar.dma_start(out=w_sb[:], in_=w[:])
                for k in range(KCH):
                    pw = psmm.tile([P, E], f32, tag="lt")
                    nc.tensor.transpose(
                        pw[:, 0:E],
                        w_sb[:, k * P : (k + 1) * P],
                        idf[0:E, 0:E],
                    )
                    nc.vector.tensor_copy(w_hi[:, k, :], pw[:, 0:E])
                    nc.vector.tensor_tensor(
                        w_lo[:, k, :], pw[:, 0:E], w_hi[:, k, :],
                        mybir.AluOpType.subtract,
                    )

            xr = x[:].rearrange("(p m b) h -> p m b h", p=P, m=NM, b=BB)
            our = out[:].rearrange("(p m b) k -> p m b k", p=P, m=NM, b=BB)

            def load_and_transpose(m, hsplit=False):
                # Loads alternate the two HWDGE rings (SP + ACT). Steady
                # state: one load per token-quarter. Megatile 0 (hsplit):
                # split along H instead, so transpose chunk k waits only on
                # h-quarter k//4 and the pipeline fills ~3us earlier.
                xq = []
                HQ = H // BB
                for c in range(BB):
                    eng = nc.sync if c % 2 == 0 else nc.scalar
                    if hsplit:
                        t = xpool.tile([P, BB, HQ], f32, tag="xin")
                        eng.dma_start(
                            out=t[:], in_=xr[:, m, :, c * HQ : (c + 1) * HQ]
                        )
                    else:
                        t = xpool.tile([P, H], f32, tag="xin")
                        eng.dma_start(out=t[:], in_=xr[:, m, c, :])
                    xq.append(t)

                def src(k, b):
                    if hsplit:
                        kq = HQ // P
                        return xq[k // kq][:, b, (k % kq) * P : (k % kq + 1) * P]
                    return xq[b][:, k * P : (k + 1) * P]

                x_hi = xhipool.tile([P, KCH, MEGA], f32r)
                x_lo = xlopool.tile([P, KCH, MEGA], f32r)
                for k in range(KCH):
                    pt = pst.tile([P, MEGA], f32, tag="pt")
                    for b in range(BB):
                        nc.tensor.transpose(
                            pt[:, b * P : (b + 1) * P],
                            src(k, b),
                            idf,
                        )
                    # hi = f32r(x) on ACT (1-input); lo = f32r(x - hi) on DVE
                    nc.scalar.copy(x_hi[:, k, :], pt[:])
                    nc.vector.tensor_tensor(
                        x_lo[:, k, :], pt[:], x_hi[:, k, :],
                        mybir.AluOpType.subtract,
                    )
                return x_hi, x_lo

            def compute(m, x_hi, x_lo, t0=0, width=MEGA):
                nb = width // P  # token blocks in this slice
                b0 = t0 // P
                # logitsT[64, width] += w_hi.x_hi + w_hi.x_lo + w_lo.x_hi
                lt = psmm.tile([E, width], f32, tag="lt")
                n_acc = 3 * KCH
                i_acc = 0
                for k in range(KCH):
                    for wt_k, xt_k in (
                        (w_hi, x_hi),
                        (w_hi, x_lo),
                        (w_lo, x_hi),
                    ):
                        nc.tensor.matmul(
                            lt[:],
                            wt_k[:, k, :],
                            xt_k[:, k, t0 : t0 + width],
                            start=(i_acc == 0),
                            stop=(i_acc == n_acc - 1),
                        )
                        i_acc += 1
                lts = ltspool.tile([E, width], f32, tag="lts")
                nc.vector.tensor_copy(lts[:], lt[:])

                # transpose logits back -> [128t, 64e] blocks in PSUM (fp32)
                lg = pslg.tile([P, nb * E], f32, tag="lg")
                for b in range(nb):
                    nc.tensor.transpose(
                        lg[:, b * E : (b + 1) * E],
                        lts[:, b * P : (b + 1) * P],
                        idf[0:E, 0:E],
                    )

                # --- top-k pipeline, all nb token-blocks fused per op ---
                BB = nb
                lg3 = lg[:].rearrange("p (b e) -> p b e", b=BB)  # [128,nb,64]
                # e = exp(logit - max): keeps ACT exp args in [-24, 0] where
                # the table is ~4x more accurate (fewer selection-flip risks
                # near group-boundary ties). Per-block bias via DVE subtract.
                nmax = spool.tile([P, BB], f32, tag="nmax")
                nc.vector.tensor_reduce(
                    nmax[:], lg3, axis=X, op=mybir.AluOpType.max, negate=True
                )
                lsub = spool.tile([P, BB, E], f32, tag="lsub")
                nc.vector.tensor_tensor(
                    lsub[:],
                    lg3,
                    nmax[:].unsqueeze(2).broadcast_to([P, BB, E]),
                    mybir.AluOpType.add,
                )
                e_sb = spool.tile([P, BB, E], f32, tag="esb")
                nc.scalar.activation(
                    e_sb[:], lsub[:], mybir.ActivationFunctionType.Exp
                )
                e4 = e_sb[:].rearrange("p b (g j) -> p b g j", g=G)
                gmax = spool.tile([P, BB, G], f32, tag="gmax")
                nc.vector.tensor_reduce(
                    gmax[:], e4, axis=X, op=mybir.AluOpType.max
                )
                gsort = spool.tile([P, BB, 8], f32, tag="gsort")
                for b in range(BB):
                    nc.vector.max(gsort[:, b, :], gmax[:, b, :])
                gmask = spool.tile([P, BB, G], f32, tag="gmask")
                nc.vector.tensor_tensor(
                    gmask[:],
                    gmax[:],
                    gsort[:, :, TG - 1 : TG].broadcast_to([P, BB, G]),
                    mybir.AluOpType.is_ge,
                )
                me = spool.tile([P, BB, E], f32, tag="me")
                nc.vector.tensor_tensor(
                    me[:].rearrange("p b (g j) -> p b g j", g=G),
                    e4,
                    gmask[:].unsqueeze(3).broadcast_to([P, BB, G, PG]),
                    mybir.AluOpType.mult,
                )
                t8 = spool.tile([P, BB, 8], f32, tag="t8")
                for b in range(BB):
                    nc.vector.max(t8[:, b, :], me[:, b, :])
                ssum = spool.tile([P, BB], f32, tag="ssum")
                nc.vector.tensor_reduce(
                    ssum[:], t8[:, :, 0:TK], axis=X, op=mybir.AluOpType.add
                )
                rec = spool.tile([P, BB], f32, tag="rec")
                nc.vector.reciprocal(rec[:], ssum[:])
                ow = opool.tile([P, BB, TK], f32, tag="ow")
                nc.vector.tensor_tensor(
                    ow[:],
                    t8[:, :, 0:TK],
                    rec[:].unsqueeze(2).broadcast_to([P, BB, TK]),
                    mybir.AluOpType.mult,
                )
                nc.sync.dma_start(out=our[:, m, b0 : b0 + nb], in_=ow[:])

            # two-stage software pipeline: transposes/copies of megatile m
            # are issued alongside the matmuls/topk of megatile m-1 so the
            # PE never waits on PSUM->SBUF copies of the tile it multiplies.
            prev = None
            w_done = False
            warm_pe()
            for _r in range(repeat):
                for m in range(NM):
                    cur = (m, *load_and_transpose(m))
                    if not w_done:
                        setup_w()
                        w_done = True
                    if prev is not None:
                        compute(*prev)
                    prev = cur
            if prev is not None:
                # split the final megatile so its top-k overlaps the second
                # half-chain instead of serializing after the last matmul
                m_l, xh_l, xl_l = prev
                compute(m_l, xh_l, xl_l, 0, MEGA // 2)
                compute(m_l, xh_l, xl_l, MEGA // 2, MEGA // 2)

    nc.compile()
    return nc


_NC_CACHE = {}


def _get_nc(t_core):
    if t_core not in _NC_CACHE:
        _NC_CACHE[t_core] = build_nc(t_core)
    return _NC_CACHE[t_core]


def run_sharded(flat_x, w, trace=False, **kw):
    """flat_x: [T, H] f32. Returns ([T, 6] f32, BassKernelResults)."""
    from concourse.bass_utils import run_bass_kernel_spmd

    T = flat_x.shape[0]
    tc = T // N_CORES
    nc = _get_nc(tc)
    in_maps = [
        {"x": np.ascontiguousarray(flat_x[i * tc : (i + 1) * tc]), "w": w}
        for i in range(N_CORES)
    ]
    res = run_bass_kernel_spmd(nc, in_maps, list(range(N_CORES)), trace=trace, **kw)
    outs = [np.asarray(res.results[i]["out"]) for i in range(N_CORES)]
    return np.concatenate(outs, axis=0), res


def kernel(hidden_states, kernel):
    hs = np.asarray(hidden_states, dtype=np.float32)
    w = np.ascontiguousarray(np.asarray(kernel, dtype=np.float32))
    B, S, Hh = hs.shape
    flat = np.ascontiguousarray(hs.reshape(B * S, Hh))
    out, _ = run_sharded(flat, w)
    return out


# revision 25
# speedup vs baseline: 2.3579x; 1.0555x over previous
"""MoE gate (DeepSeek-V2 style, group-limited greedy top-k) for Trainium2.

Full-input contract: kernel(hidden_states[4,8192,2048] f32, kernel[64,2048] f32)
-> topk_weight [32768, 6] f32.

Strategy: pure data-parallel over 8 NeuronCores (4096 tokens each).
Per core:
  - tokens are remapped so partition p owns a contiguous 32-token DRAM range
    (t = p*32 + m*4 + b), making every DMA descriptor large & contiguous.
  - per 512-token megatile: DMA x -> SBUF [128, 4, 2048]; PE-transpose
    (float32r mode, exact fp32 bits) into PSUM; copy PSUM->SBUF xT
    [128h, 512t] alternating ACT/DVE engines; accumulate logitsT[64, 512]
    over 16 h-chunks with float32r matmuls (W stationary); PE-transpose
    logits back to [128t, 64e]; then a per-128-token top-k pipeline on
    DVE/ACT using the hardware top-8 sort (InstMax):
      softmax denominator cancels in the final normalization, so we only
      need e = exp(logit - max); group-max -> sort -> 3rd value threshold
      -> group mask -> masked e -> top-8 sort -> sum top-6 -> reciprocal
      -> scale.
"""

import sys

if "/opt/trn_rl_repo" not in sys.path:
    sys.path.insert(0, "/opt/trn_rl_repo")

import numpy as np

# Problem constants (hardcoded per contract)
N_CORES = 8
H = 2048
E = 64  # n_routed_experts
G = 8  # n_group
PG = E // G  # experts per group
TG = 3  # topk_group
TK = 6  # top_k
P = 128  # partitions
MEGA = 512  # tokens per megatile
BB = MEGA // P  # 4 token blocks per megatile
KCH = H // P  # 16 contraction chunks


def build_nc(t_core, repeat=1):
    """Build the single-core Bass program for a t_core-token shard.

    repeat>1 re-runs the whole pipeline (timing experiments only).
    """
    from concourse import bacc, mybir, masks
    from concourse.tile import TileContext

    f32 = mybir.dt.float32
    f32r = mybir.dt.float32r
    f8 = mybir.dt.float8e4
    X = mybir.AxisListType.X
    NM = t_core // MEGA
    assert t_core % MEGA == 0

    nc = bacc.Bacc()
    x = nc.declare_dram_parameter("x", [t_core, H], f32, isOutput=False)
    w = nc.declare_dram_parameter("w", [E, H], f32, isOutput=False)
    out = nc.declare_dram_parameter("out", [t_core, TK], f32, isOutput=True)

    with TileContext(nc) as tc:
        with (
            tc.tile_pool(name="const", bufs=1) as cpool,
            tc.tile_pool(name="xin", bufs=6) as xpool,
            tc.tile_pool(name="xhi", bufs=2) as xhipool,
            tc.tile_pool(name="xlo", bufs=2) as xlopool,
            tc.tile_pool(name="lts", bufs=2) as ltspool,
            tc.tile_pool(name="small", bufs=2) as spool,
            tc.tile_pool(name="outp", bufs=2) as opool,
            tc.tile_pool(name="ps_t", bufs=5, space="PSUM") as pst,
            tc.tile_pool(name="ps_mm", bufs=2, space="PSUM") as psmm,
            tc.tile_pool(name="ps_lg", bufs=1, space="PSUM") as pslg,
        ):
            identf = cpool.tile([P, P], f32)
            masks.make_identity(nc, identf[:])
            idf = identf[:]

            w_sb = cpool.tile([E, H], f32)
            w_hi = cpool.tile([P, KCH, E], f32r)
            w_lo = cpool.tile([P, KCH, E], f32r)
            w_hi8 = cpool.tile([P, KCH, E], f8)

            def warm_pe(n=24):
                # Dummy identity transposes fill the otherwise-idle DMA head
                # and burn through the PE p-state ramp (P3/HAM warmup), so
                # real transposes start at full clock.
                pwm = pslg.tile([P, P], f32, tag="lg")
                for _ in range(n):
                    nc.tensor.transpose(pwm[:], idf, idf)

            def setup_w():
                # W: load + transpose once -> w_hi/w_lo [128h, k, 64e] f32r
                # (hi/lo split so that 3 f32r matmuls reach fp32 accuracy).
                # Issued after megatile 0's loads so it doesn't gate the head;
                # chunked so the first W transposes start early.
                nc.scalar.dma_start(out=w_sb[:], in_=w[:])
                for k in range(KCH):
                    pw = psmm.tile([P, E], f32, tag="lt")
                    nc.tensor.transpose(
                        pw[:, 0:E],
                        w_sb[:, k * P : (k + 1) * P],
                        idf[0:E, 0:E],
                    )
                    nc.vector.tensor_copy(w_hi[:, k, :], pw[:, 0:E])
                    nc.vector.tensor_tensor(
                        w_lo[:, k, :], pw[:, 0:E], w_hi[:, k, :],
                        mybir.AluOpType.subtract,
                    )
                    nc.vector.tensor_copy(w_hi8[:, k, :], w_hi[:, k, :])

            xr = x[:].rearrange("(p m b) h -> p m b h", p=P, m=NM, b=BB)
            our = out[:].rearrange("(p m b) k -> p m b k", p=P, m=NM, b=BB)

            def load_and_transpose(m, hsplit=False):
                # Loads alternate the two HWDGE rings (SP + ACT). Steady
                # state: one load per token-quarter. Megatile 0 (hsplit):
                # split along H instead, so transpose chunk k waits only on
                # h-quarter k//4 and the pipeline fills ~3us earlier.
                xq = []
                HQ = H // BB
                for c in range(BB):
                    eng = nc.sync if c % 2 == 0 else nc.scalar
                    if hsplit:
                        t = xpool.tile([P, BB, HQ], f32, tag="xin")
                        eng.dma_start(
                            out=t[:], in_=xr[:, m, :, c * HQ : (c + 1) * HQ]
                        )
                    else:
                        t = xpool.tile([P, H], f32, tag="xin")
                        eng.dma_start(out=t[:], in_=xr[:, m, c, :])
                    xq.append(t)

                def src(k, b):
                    if hsplit:
                        kq = HQ // P
                        return xq[k // kq][:, b, (k % kq) * P : (k % kq + 1) * P]
                    return xq[b][:, k * P : (k + 1) * P]

                x_hi = xhipool.tile([P, KCH, MEGA], f32r)
                x_lo = xlopool.tile([P, KCH, MEGA], f8)
                for k in range(KCH):
                    pt = pst.tile([P, MEGA], f32, tag="pt")
                    for b in range(BB):
                        nc.tensor.transpose(
                            pt[:, b * P : (b + 1) * P],
                            src(k, b),
                            idf,
                        )
                    # hi = f32r(4096*x) on ACT (power-of-2 scale: exact);
                    # lo = fp8(4096*x - hi) on DVE. The 4096 scale puts lo in
                    # fp8e4m3's normal range; all three matmul passes then
                    # share the same 2^12 scale in one PSUM group, descaled
                    # for free by the exp activation's scale parameter.
                    nc.scalar.mul(x_hi[:, k, :], pt[:], 4096.0)
                    nc.vector.scalar_tensor_tensor(
                        x_lo[:, k, :], pt[:], 4096.0, x_hi[:, k, :],
                        mybir.AluOpType.mult, mybir.AluOpType.subtract,
                    )
                return x_hi, x_lo

            def compute(m, x_hi, x_lo, t0=0, width=MEGA):
                nb = width // P  # token blocks in this slice
                b0 = t0 // P
                # logitsT[64, width] += w_hi.x_hi + w_hi.x_lo + w_lo.x_hi
                lt = psmm.tile([E, width], f32, tag="lt")
                n_acc = 2 * KCH + KCH // 2
                i_acc = 0
                for k in range(KCH):
                    for wt_k, xt_k in ((w_hi, x_hi), (w_lo, x_hi)):
                        nc.tensor.matmul(
                            lt[:],
                            wt_k[:, k, :],
                            xt_k[:, k, t0 : t0 + width],
                            start=(i_acc == 0),
                            stop=(i_acc == n_acc - 1),
                        )
                        i_acc += 1
                # cross term w_hi.x_lo in fp8 DoubleRow: each matmul
                # contracts TWO 128-h chunks (0.5 cyc/row)
                for p in range(KCH // 2):
                    nc.tensor.matmul(
                        lt[:],
                        w_hi8[:, 2 * p : 2 * p + 2, :],
                        x_lo[:, 2 * p : 2 * p + 2, t0 : t0 + width],
                        start=False,
                        stop=(i_acc == n_acc - 1),
                        perf_mode=mybir.MatmulPerfMode.DoubleRow,
                    )
                    i_acc += 1
                lts = ltspool.tile([E, width], f32, tag="lts")
                nc.vector.tensor_copy(lts[:], lt[:])

                # transpose logits back -> [128t, 64e] blocks in PSUM (fp32)
                lg = pslg.tile([P, nb * E], f32, tag="lg")
                for b in range(nb):
                    nc.tensor.transpose(
                        lg[:, b * E : (b + 1) * E],
                        lts[:, b * P : (b + 1) * P],
                        idf[0:E, 0:E],
                    )

                # --- top-k pipeline, all nb token-blocks fused per op ---
                BB = nb
                lg3 = lg[:].rearrange("p (b e) -> p b e", b=BB)  # [128,nb,64]
                # e = exp(logit - max): keeps ACT exp args in [-24, 0] where
                # the table is ~4x more accurate (fewer selection-flip risks
                # near group-boundary ties). Per-block bias via DVE subtract.
                nmax = spool.tile([P, BB], f32, tag="nmax")
                nc.vector.tensor_reduce(
                    nmax[:], lg3, axis=X, op=mybir.AluOpType.max, negate=True
                )
                lsub = spool.tile([P, BB, E], f32, tag="lsub")
                nc.vector.tensor_tensor(
                    lsub[:],
                    lg3,
                    nmax[:].unsqueeze(2).broadcast_to([P, BB, E]),
                    mybir.AluOpType.add,
                )
                e_sb = spool.tile([P, BB, E], f32, tag="esb")
                nc.scalar.activation(
                    e_sb[:], lsub[:], mybir.ActivationFunctionType.Exp,
                    scale=2.0 ** -12,
                )
                e4 = e_sb[:].rearrange("p b (g j) -> p b g j", g=G)
                gmax = spool.tile([P, BB, G], f32, tag="gmax")
                nc.vector.tensor_reduce(
                    gmax[:], e4, axis=X, op=mybir.AluOpType.max
                )
                gsort = spool.tile([P, BB, 8], f32, tag="gsort")
                for b in range(BB):
                    nc.vector.max(gsort[:, b, :], gmax[:, b, :])
                gmask = spool.tile([P, BB, G], f32, tag="gmask")
                nc.vector.tensor_tensor(
                    gmask[:],
                    gmax[:],
                    gsort[:, :, TG - 1 : TG].broadcast_to([P, BB, G]),
                    mybir.AluOpType.is_ge,
                )
                me = spool.tile([P, BB, E], f32, tag="me")
                nc.vector.tensor_tensor(
                    me[:].rearrange("p b (g j) -> p b g j", g=G),
                    e4,
                    gmask[:].unsqueeze(3).broadcast_to([P, BB, G, PG]),
                    mybir.AluOpType.mult,
                )
                t8 = spool.tile([P, BB, 8], f32, tag="t8")
                for b in range(BB):
                    nc.vector.max(t8[:, b, :], me[:, b, :])
                ssum = spool.tile([P, BB], f32, tag="ssum")
                nc.vector.tensor_reduce(
                    ssum[:], t8[:, :, 0:TK], axis=X, op=mybir.AluOpType.add
                )
                rec = spool.tile([P, BB], f32, tag="rec")
                nc.vector.reciprocal(rec[:], ssum[:])
                ow = opool.tile([P, BB, TK], f32, tag="ow")
                nc.vector.tensor_tensor(
                    ow[:],
                    t8[:, :, 0:TK],
                    rec[:].unsqueeze(2).broadcast_to([P, BB, TK]),
                    mybir.AluOpType.mult,
                )
                nc.sync.dma_start(out=our[:, m, b0 : b0 + nb], in_=ow[:])

            # two-stage software pipeline: transposes/copies of megatile m
            # are issued alongside the matmuls/topk of megatile m-1 so the
            # PE never waits on PSUM->SBUF copies of the tile it multiplies.
            prev = None
            w_done = False
            warm_pe()
            for _r in range(repeat):
                for m in range(NM):
                    cur = (m, *load_and_transpose(m))
                    if not w_done:
                        setup_w()
                        w_done = True
                    if prev is not None:
                        compute(*prev)
                    prev = cur
            if prev is not None:
                # split the final megatile so its top-k overlaps the second
                # half-chain instead of serializing after the last matmul
                m_l, xh_l, xl_l = prev
                compute(m_l, xh_l, xl_l, 0, MEGA // 2)
                compute(m_l, xh_l, xl_l, MEGA // 2, MEGA // 2)

    nc.compile()
    return nc


_NC_CACHE = {}


def _get_nc(t_core):
    if t_core not in _NC_CACHE:
        _NC_CACHE[t_core] = build_nc(t_core)
    return _NC_CACHE[t_core]


def run_sharded(flat_x, w, trace=False, **kw):
    """flat_x: [T, H] f32. Returns ([T, 6] f32, BassKernelResults)."""
    from concourse.bass_utils import run_bass_kernel_spmd

    T = flat_x.shape[0]
    tc = T // N_CORES
    nc = _get_nc(tc)
    in_maps = [
        {"x": np.ascontiguousarray(flat_x[i * tc : (i + 1) * tc]), "w": w}
        for i in range(N_CORES)
    ]
    res = run_bass_kernel_spmd(nc, in_maps, list(range(N_CORES)), trace=trace, **kw)
    outs = [np.asarray(res.results[i]["out"]) for i in range(N_CORES)]
    return np.concatenate(outs, axis=0), res


def kernel(hidden_states, kernel):
    hs = np.asarray(hidden_states, dtype=np.float32)
    w = np.ascontiguousarray(np.asarray(kernel, dtype=np.float32))
    B, S, Hh = hs.shape
    flat = np.ascontiguousarray(hs.reshape(B * S, Hh))
    out, _ = run_sharded(flat, w)
    return out


# revision 32
# speedup vs baseline: 2.5300x; 1.0730x over previous
"""MoE gate (DeepSeek-V2 style, group-limited greedy top-k) for Trainium2.

Full-input contract: kernel(hidden_states[4,8192,2048] f32, kernel[64,2048] f32)
-> topk_weight [32768, 6] f32.

Strategy: pure data-parallel over 8 NeuronCores (4096 tokens each).
Per core:
  - tokens are remapped so partition p owns a contiguous 32-token DRAM range
    (t = p*32 + m*4 + b), making every DMA descriptor large & contiguous.
  - per 512-token megatile: DMA x -> SBUF [128, 4, 2048]; PE-transpose
    (float32r mode, exact fp32 bits) into PSUM; copy PSUM->SBUF xT
    [128h, 512t] alternating ACT/DVE engines; accumulate logitsT[64, 512]
    over 16 h-chunks with float32r matmuls (W stationary); PE-transpose
    logits back to [128t, 64e]; then a per-128-token top-k pipeline on
    DVE/ACT using the hardware top-8 sort (InstMax):
      softmax denominator cancels in the final normalization, so we only
      need e = exp(logit - max); group-max -> sort -> 3rd value threshold
      -> group mask -> masked e -> top-8 sort -> sum top-6 -> reciprocal
      -> scale.
"""

import sys

if "/opt/trn_rl_repo" not in sys.path:
    sys.path.insert(0, "/opt/trn_rl_repo")

import numpy as np

# Problem constants (hardcoded per contract)
N_CORES = 8
H = 2048
E = 64  # n_routed_experts
G = 8  # n_group
PG = E // G  # experts per group
TG = 3  # topk_group
TK = 6  # top_k
P = 128  # partitions
MEGA = 512  # tokens per megatile
BB = MEGA // P  # 4 token blocks per megatile
KCH = H // P  # 16 contraction chunks


def build_nc(t_core, repeat=1):
    """Build the single-core Bass program for a t_core-token shard.

    repeat>1 re-runs the whole pipeline (timing experiments only).
    """
    from concourse import bacc, mybir, masks
    from concourse.tile import TileContext

    f32 = mybir.dt.float32
    f32r = mybir.dt.float32r
    f8 = mybir.dt.float8e4
    X = mybir.AxisListType.X
    NM = t_core // MEGA
    assert t_core % MEGA == 0

    nc = bacc.Bacc()
    x = nc.declare_dram_parameter("x", [t_core, H], f32, isOutput=False)
    w = nc.declare_dram_parameter("w", [E, H], f32, isOutput=False)
    out = nc.declare_dram_parameter("out", [t_core, TK], f32, isOutput=True)

    with TileContext(nc) as tc:
        with (
            tc.tile_pool(name="const", bufs=1) as cpool,
            tc.tile_pool(name="xin", bufs=6) as xpool,
            tc.tile_pool(name="xhi", bufs=2) as xhipool,
            tc.tile_pool(name="xlo", bufs=2) as xlopool,
            tc.tile_pool(name="lts", bufs=2) as ltspool,
            tc.tile_pool(name="small", bufs=2) as spool,
            tc.tile_pool(name="outp", bufs=2) as opool,
            tc.tile_pool(name="ps_t", bufs=5, space="PSUM") as pst,
            tc.tile_pool(name="ps_mm", bufs=2, space="PSUM") as psmm,
            tc.tile_pool(name="ps_lg", bufs=1, space="PSUM") as pslg,
        ):
            identf = cpool.tile([P, P], f32)
            masks.make_identity(nc, identf[:])
            idf = identf[:]

            w_sb = cpool.tile([E, H], f32)
            w_hi = cpool.tile([P, KCH, E], f32r)
            w_lo = cpool.tile([P, KCH, E], f32r)
            w_hi8 = cpool.tile([P, KCH, E], f8)

            def warm_pe(n=24):
                # Dummy identity transposes fill the otherwise-idle DMA head
                # and burn through the PE p-state ramp (P3/HAM warmup), so
                # real transposes start at full clock.
                pwm = pslg.tile([P, P], f32, tag="lg")
                for _ in range(n):
                    nc.tensor.transpose(pwm[:], idf, idf)

            def setup_w():
                # W: load + transpose once -> w_hi/w_lo [128h, k, 64e] f32r
                # (hi/lo split so that 3 f32r matmuls reach fp32 accuracy).
                # Issued after megatile 0's loads so it doesn't gate the head;
                # chunked so the first W transposes start early.
                nc.scalar.dma_start(out=w_sb[:], in_=w[:])
                for k in range(KCH):
                    pw = psmm.tile([P, E], f32, tag="lt")
                    nc.tensor.transpose(
                        pw[:, 0:E],
                        w_sb[:, k * P : (k + 1) * P],
                        idf[0:E, 0:E],
                    )
                    nc.vector.tensor_copy(w_hi[:, k, :], pw[:, 0:E])
                    nc.vector.tensor_tensor(
                        w_lo[:, k, :], pw[:, 0:E], w_hi[:, k, :],
                        mybir.AluOpType.subtract,
                    )
                    nc.vector.tensor_copy(w_hi8[:, k, :], w_hi[:, k, :])

            xr = x[:].rearrange("(p m b) h -> p m b h", p=P, m=NM, b=BB)
            our = out[:].rearrange("(p m b) k -> p m b k", p=P, m=NM, b=BB)

            def load_and_transpose(m, hsplit=False):
                # Loads alternate the two HWDGE rings (SP + ACT). Steady
                # state: one load per token-quarter. Megatile 0 (hsplit):
                # split along H instead, so transpose chunk k waits only on
                # h-quarter k//4 and the pipeline fills ~3us earlier.
                xq = []
                HQ = H // BB
                for c in range(BB):
                    eng = nc.sync if c < 3 else nc.scalar
                    if hsplit:
                        t = xpool.tile([P, BB, HQ], f32, tag="xin")
                        eng.dma_start(
                            out=t[:], in_=xr[:, m, :, c * HQ : (c + 1) * HQ]
                        )
                    else:
                        t = xpool.tile([P, H], f32, tag="xin")
                        eng.dma_start(out=t[:], in_=xr[:, m, c, :])
                    xq.append(t)

                def src(k, b):
                    if hsplit:
                        kq = HQ // P
                        return xq[k // kq][:, b, (k % kq) * P : (k % kq + 1) * P]
                    return xq[b][:, k * P : (k + 1) * P]

                x_hi = xhipool.tile([P, KCH, MEGA], f32r)
                x_lo = xlopool.tile([P, KCH, MEGA], f8)
                for k in range(KCH):
                    pt = pst.tile([P, MEGA], f32, tag="pt")
                    for b in range(BB):
                        nc.tensor.transpose(
                            pt[:, b * P : (b + 1) * P],
                            src(k, b),
                            idf,
                        )
                    # hi = f32r(4096*x) on ACT (power-of-2 scale: exact);
                    # lo = fp8(4096*x - hi) on DVE. The 4096 scale puts lo in
                    # fp8e4m3's normal range; all three matmul passes then
                    # share the same 2^12 scale in one PSUM group, descaled
                    # for free by the exp activation's scale parameter.
                    nc.scalar.mul(x_hi[:, k, :], pt[:], 4096.0)
                    nc.vector.scalar_tensor_tensor(
                        x_lo[:, k, :], pt[:], 4096.0, x_hi[:, k, :],
                        mybir.AluOpType.mult, mybir.AluOpType.subtract,
                    )
                return x_hi, x_lo

            def compute(m, x_hi, x_lo, t0=0, width=MEGA):
                nb = width // P  # token blocks in this slice
                b0 = t0 // P
                # logitsT[64, width] += w_hi.x_hi + w_hi.x_lo + w_lo.x_hi
                lt = psmm.tile([E, width], f32, tag="lt")
                n_acc = 2 * KCH + KCH // 2
                i_acc = 0
                for k in range(KCH):
                    for wt_k, xt_k in ((w_hi, x_hi), (w_lo, x_hi)):
                        nc.tensor.matmul(
                            lt[:],
                            wt_k[:, k, :],
                            xt_k[:, k, t0 : t0 + width],
                            start=(i_acc == 0),
                            stop=(i_acc == n_acc - 1),
                        )
                        i_acc += 1
                # cross term w_hi.x_lo in fp8 DoubleRow: each matmul
                # contracts TWO 128-h chunks (0.5 cyc/row)
                for p in range(KCH // 2):
                    nc.tensor.matmul(
                        lt[:],
                        w_hi8[:, 2 * p : 2 * p + 2, :],
                        x_lo[:, 2 * p : 2 * p + 2, t0 : t0 + width],
                        start=False,
                        stop=(i_acc == n_acc - 1),
                        perf_mode=mybir.MatmulPerfMode.DoubleRow,
                    )
                    i_acc += 1
                lts = ltspool.tile([E, width], f32, tag="lts")
                nc.vector.tensor_copy(lts[:], lt[:])

                # transpose logits back -> [128t, 64e] blocks in PSUM (fp32)
                lg = pslg.tile([P, nb * E], f32, tag="lg")
                for b in range(nb):
                    nc.tensor.transpose(
                        lg[:, b * E : (b + 1) * E],
                        lts[:, b * P : (b + 1) * P],
                        idf[0:E, 0:E],
                    )

                # --- top-k pipeline, all nb token-blocks fused per op ---
                BB = nb
                lg3 = lg[:].rearrange("p (b e) -> p b e", b=BB)  # [128,nb,64]
                # e = exp(logit - max): keeps ACT exp args in [-24, 0] where
                # the table is ~4x more accurate (fewer selection-flip risks
                # near group-boundary ties). Per-block bias via DVE subtract.
                nmax = spool.tile([P, BB], f32, tag="nmax")
                nc.vector.tensor_reduce(
                    nmax[:], lg3, axis=X, op=mybir.AluOpType.max, negate=True
                )
                lsub = spool.tile([P, BB, E], f32, tag="lsub")
                nc.vector.tensor_tensor(
                    lsub[:],
                    lg3,
                    nmax[:].unsqueeze(2).broadcast_to([P, BB, E]),
                    mybir.AluOpType.add,
                )
                e_sb = spool.tile([P, BB, E], f32, tag="esb")
                nc.scalar.activation(
                    e_sb[:], lsub[:], mybir.ActivationFunctionType.Exp,
                    scale=2.0 ** -12,
                )
                e4 = e_sb[:].rearrange("p b (g j) -> p b g j", g=G)
                gmax = spool.tile([P, BB, G], f32, tag="gmax")
                nc.vector.tensor_reduce(
                    gmax[:], e4, axis=X, op=mybir.AluOpType.max
                )
                gsort = spool.tile([P, BB, 8], f32, tag="gsort")
                for b in range(BB):
                    nc.vector.max(gsort[:, b, :], gmax[:, b, :])
                gmask = spool.tile([P, BB, G], f32, tag="gmask")
                nc.vector.tensor_tensor(
                    gmask[:],
                    gmax[:],
                    gsort[:, :, TG - 1 : TG].broadcast_to([P, BB, G]),
                    mybir.AluOpType.is_ge,
                )
                me = spool.tile([P, BB, E], f32, tag="me")
                nc.vector.tensor_tensor(
                    me[:].rearrange("p b (g j) -> p b g j", g=G),
                    e4,
                    gmask[:].unsqueeze(3).broadcast_to([P, BB, G, PG]),
                    mybir.AluOpType.mult,
                )
                t8 = spool.tile([P, BB, 8], f32, tag="t8")
                for b in range(BB):
                    nc.vector.max(t8[:, b, :], me[:, b, :])
                ssum = spool.tile([P, BB], f32, tag="ssum")
                nc.vector.tensor_reduce(
                    ssum[:], t8[:, :, 0:TK], axis=X, op=mybir.AluOpType.add
                )
                rec = spool.tile([P, BB], f32, tag="rec")
                nc.vector.reciprocal(rec[:], ssum[:])
                ow = opool.tile([P, BB, TK], f32, tag="ow")
                nc.vector.tensor_tensor(
                    ow[:],
                    t8[:, :, 0:TK],
                    rec[:].unsqueeze(2).broadcast_to([P, BB, TK]),
                    mybir.AluOpType.mult,
                )
                nc.sync.dma_start(out=our[:, m, b0 : b0 + nb], in_=ow[:])

            # two-stage software pipeline: transposes/copies of megatile m
            # are issued alongside the matmuls/topk of megatile m-1 so the
            # PE never waits on PSUM->SBUF copies of the tile it multiplies.
            prev = None
            w_done = False
            warm_pe()
            for _r in range(repeat):
                for m in range(NM):
                    cur = (m, *load_and_transpose(m))
                    if not w_done:
                        setup_w()
                        w_done = True
                    if prev is not None:
                        compute(*prev)
                    prev = cur
            if prev is not None:
                # split the final megatile so its top-k overlaps the second
                # half-chain instead of serializing after the last matmul
                m_l, xh_l, xl_l = prev
                compute(m_l, xh_l, xl_l, 0, MEGA // 2)
                compute(m_l, xh_l, xl_l, MEGA // 2, MEGA // 2)

    nc.compile()
    return nc


_NC_CACHE = {}


def _get_nc(t_core):
    if t_core not in _NC_CACHE:
        _NC_CACHE[t_core] = build_nc(t_core)
    return _NC_CACHE[t_core]


def run_sharded(flat_x, w, trace=False, **kw):
    """flat_x: [T, H] f32. Returns ([T, 6] f32, BassKernelResults)."""
    from concourse.bass_utils import run_bass_kernel_spmd

    T = flat_x.shape[0]
    tc = T // N_CORES
    nc = _get_nc(tc)
    in_maps = [
        {"x": np.ascontiguousarray(flat_x[i * tc : (i + 1) * tc]), "w": w}
        for i in range(N_CORES)
    ]
    res = run_bass_kernel_spmd(nc, in_maps, list(range(N_CORES)), trace=trace, **kw)
    outs = [np.asarray(res.results[i]["out"]) for i in range(N_CORES)]
    return np.concatenate(outs, axis=0), res


def kernel(hidden_states, kernel):
    hs = np.asarray(hidden_states, dtype=np.float32)
    w = np.ascontiguousarray(np.asarray(kernel, dtype=np.float32))
    B, S, Hh = hs.shape
    flat = np.ascontiguousarray(hs.reshape(B * S, Hh))
    out, _ = run_sharded(flat, w)
    return out


# revision 40
# speedup vs baseline: 2.5392x; 1.0036x over previous
"""MoE gate (DeepSeek-V2 style, group-limited greedy top-k) for Trainium2.

Full-input contract: kernel(hidden_states[4,8192,2048] f32, kernel[64,2048] f32)
-> topk_weight [32768, 6] f32.

Strategy: pure data-parallel over 8 NeuronCores (4096 tokens each).
Per core:
  - tokens are remapped so partition p owns a contiguous 32-token DRAM range
    (t = p*32 + m*4 + b), making every DMA descriptor large & contiguous.
  - per 512-token megatile: DMA x -> SBUF [128, 4, 2048]; PE-transpose
    (float32r mode, exact fp32 bits) into PSUM; copy PSUM->SBUF xT
    [128h, 512t] alternating ACT/DVE engines; accumulate logitsT[64, 512]
    over 16 h-chunks with float32r matmuls (W stationary); PE-transpose
    logits back to [128t, 64e]; then a per-128-token top-k pipeline on
    DVE/ACT using the hardware top-8 sort (InstMax):
      softmax denominator cancels in the final normalization, so we only
      need e = exp(logit - max); group-max -> sort -> 3rd value threshold
      -> group mask -> masked e -> top-8 sort -> sum top-6 -> reciprocal
      -> scale.
"""

import sys

if "/opt/trn_rl_repo" not in sys.path:
    sys.path.insert(0, "/opt/trn_rl_repo")

import numpy as np

# Problem constants (hardcoded per contract)
N_CORES = 8
H = 2048
E = 64  # n_routed_experts
G = 8  # n_group
PG = E // G  # experts per group
TG = 3  # topk_group
TK = 6  # top_k
P = 128  # partitions
MEGA = 512  # tokens per megatile
BB = MEGA // P  # 4 token blocks per megatile
KCH = H // P  # 16 contraction chunks


def build_nc(t_core, repeat=1):
    """Build the single-core Bass program for a t_core-token shard.

    repeat>1 re-runs the whole pipeline (timing experiments only).
    """
    from concourse import bacc, mybir, masks
    from concourse.tile import TileContext

    f32 = mybir.dt.float32
    f32r = mybir.dt.float32r
    f8 = mybir.dt.float8e4
    X = mybir.AxisListType.X
    NM = t_core // MEGA
    assert t_core % MEGA == 0

    nc = bacc.Bacc()
    x = nc.declare_dram_parameter("x", [t_core, H], f32, isOutput=False)
    w = nc.declare_dram_parameter("w", [E, H], f32, isOutput=False)
    out = nc.declare_dram_parameter("out", [t_core, TK], f32, isOutput=True)

    with TileContext(nc) as tc:
        with (
            tc.tile_pool(name="const", bufs=1) as cpool,
            tc.tile_pool(name="xin", bufs=8) as xpool,
            tc.tile_pool(name="xhi", bufs=2) as xhipool,
            tc.tile_pool(name="xlo", bufs=2) as xlopool,
            tc.tile_pool(name="lts", bufs=3) as ltspool,
            tc.tile_pool(name="small", bufs=3) as spool,
            tc.tile_pool(name="outp", bufs=2) as opool,
            tc.tile_pool(name="ps_t", bufs=5, space="PSUM") as pst,
            tc.tile_pool(name="ps_mm", bufs=2, space="PSUM") as psmm,
            tc.tile_pool(name="ps_lg", bufs=1, space="PSUM") as pslg,
        ):
            identf = cpool.tile([P, P], f32)
            masks.make_identity(nc, identf[:])
            idf = identf[:]

            w_sb = cpool.tile([E, H], f32)
            w_hi = cpool.tile([P, KCH, E], f32r)
            w_lo = cpool.tile([P, KCH, E], f32r)
            w_hi8 = cpool.tile([P, KCH, E], f8)

            def warm_pe(n=24):
                # Dummy identity transposes fill the otherwise-idle DMA head
                # and burn through the PE p-state ramp (P3/HAM warmup), so
                # real transposes start at full clock.
                pwm = pslg.tile([P, P], f32, tag="lg")
                for _ in range(n):
                    nc.tensor.transpose(pwm[:], idf, idf)

            def setup_w():
                # W: load + transpose once -> w_hi/w_lo [128h, k, 64e] f32r
                # (hi/lo split so that 3 f32r matmuls reach fp32 accuracy).
                # Issued after megatile 0's loads so it doesn't gate the head;
                # chunked so the first W transposes start early.
                nc.scalar.dma_start(out=w_sb[:], in_=w[:])
                for k in range(KCH):
                    pw = psmm.tile([P, E], f32, tag="lt")
                    nc.tensor.transpose(
                        pw[:, 0:E],
                        w_sb[:, k * P : (k + 1) * P],
                        idf[0:E, 0:E],
                    )
                    nc.vector.tensor_copy(w_hi[:, k, :], pw[:, 0:E])
                    nc.vector.tensor_tensor(
                        w_lo[:, k, :], pw[:, 0:E], w_hi[:, k, :],
                        mybir.AluOpType.subtract,
                    )
                    nc.vector.tensor_copy(w_hi8[:, k, :], w_hi[:, k, :])

            xr = x[:].rearrange("(p m b) h -> p m b h", p=P, m=NM, b=BB)
            our = out[:].rearrange("(p m b) k -> p m b k", p=P, m=NM, b=BB)

            def load_and_transpose(m, hsplit=False):
                # Loads alternate the two HWDGE rings (SP + ACT). Steady
                # state: one load per token-quarter. Megatile 0 (hsplit):
                # split along H instead, so transpose chunk k waits only on
                # h-quarter k//4 and the pipeline fills ~3us earlier.
                xq = []
                HQ = H // BB
                for c in range(BB):
                    eng = nc.sync if c < 3 else nc.scalar
                    if hsplit:
                        t = xpool.tile([P, BB, HQ], f32, tag="xin")
                        eng.dma_start(
                            out=t[:], in_=xr[:, m, :, c * HQ : (c + 1) * HQ]
                        )
                    else:
                        t = xpool.tile([P, H], f32, tag="xin")
                        eng.dma_start(out=t[:], in_=xr[:, m, c, :])
                    xq.append(t)

                def src(k, b):
                    if hsplit:
                        kq = HQ // P
                        return xq[k // kq][:, b, (k % kq) * P : (k % kq + 1) * P]
                    return xq[b][:, k * P : (k + 1) * P]

                x_hi = xhipool.tile([P, KCH, MEGA], f32r)
                x_lo = xlopool.tile([P, KCH, MEGA], f8)
                for k in range(KCH):
                    pt = pst.tile([P, MEGA], f32, tag="pt")
                    for b in range(BB):
                        nc.tensor.transpose(
                            pt[:, b * P : (b + 1) * P],
                            src(k, b),
                            idf,
                        )
                    # hi = f32r(4096*x) on ACT (power-of-2 scale: exact);
                    # lo = fp8(4096*x - hi) on DVE. The 4096 scale puts lo in
                    # fp8e4m3's normal range; all three matmul passes then
                    # share the same 2^12 scale in one PSUM group, descaled
                    # for free by the exp activation's scale parameter.
                    nc.scalar.mul(x_hi[:, k, :], pt[:], 4096.0)
                    nc.vector.scalar_tensor_tensor(
                        x_lo[:, k, :], pt[:], 4096.0, x_hi[:, k, :],
                        mybir.AluOpType.mult, mybir.AluOpType.subtract,
                    )
                return x_hi, x_lo

            def compute(m, x_hi, x_lo, t0=0, width=MEGA):
                nb = width // P  # token blocks in this slice
                b0 = t0 // P
                # logitsT[64, width] += w_hi.x_hi + w_hi.x_lo + w_lo.x_hi
                lt = psmm.tile([E, width], f32, tag="lt")
                n_acc = 2 * KCH + KCH // 2
                i_acc = 0
                for k in range(KCH):
                    for wt_k, xt_k in ((w_hi, x_hi), (w_lo, x_hi)):
                        nc.tensor.matmul(
                            lt[:],
                            wt_k[:, k, :],
                            xt_k[:, k, t0 : t0 + width],
                            start=(i_acc == 0),
                            stop=(i_acc == n_acc - 1),
                        )
                        i_acc += 1
                # cross term w_hi.x_lo in fp8 DoubleRow: each matmul
                # contracts TWO 128-h chunks (0.5 cyc/row)
                for p in range(KCH // 2):
                    nc.tensor.matmul(
                        lt[:],
                        w_hi8[:, 2 * p : 2 * p + 2, :],
                        x_lo[:, 2 * p : 2 * p + 2, t0 : t0 + width],
                        start=False,
                        stop=(i_acc == n_acc - 1),
                        perf_mode=mybir.MatmulPerfMode.DoubleRow,
                    )
                    i_acc += 1
                lts = ltspool.tile([E, width], f32, tag="lts")
                nc.vector.tensor_copy(lts[:], lt[:])

                # transpose logits back -> [128t, 64e] blocks in PSUM (fp32)
                lg = pslg.tile([P, nb * E], f32, tag="lg")
                for b in range(nb):
                    nc.tensor.transpose(
                        lg[:, b * E : (b + 1) * E],
                        lts[:, b * P : (b + 1) * P],
                        idf[0:E, 0:E],
                    )

                # --- top-k pipeline, all nb token-blocks fused per op ---
                BB = nb
                lg3 = lg[:].rearrange("p (b e) -> p b e", b=BB)  # [128,nb,64]
                # e = exp(logit - max): keeps ACT exp args in [-24, 0] where
                # the table is ~4x more accurate (fewer selection-flip risks
                # near group-boundary ties). Per-block bias via DVE subtract.
                nmax = spool.tile([P, BB], f32, tag="nmax")
                nc.vector.tensor_reduce(
                    nmax[:], lg3, axis=X, op=mybir.AluOpType.max, negate=True
                )
                lsub = spool.tile([P, BB, E], f32, tag="lsub")
                nc.vector.tensor_tensor(
                    lsub[:],
                    lg3,
                    nmax[:].unsqueeze(2).broadcast_to([P, BB, E]),
                    mybir.AluOpType.add,
                )
                e_sb = spool.tile([P, BB, E], f32, tag="esb")
                nc.scalar.activation(
                    e_sb[:], lsub[:], mybir.ActivationFunctionType.Exp,
                    scale=2.0 ** -12,
                )
                e4 = e_sb[:].rearrange("p b (g j) -> p b g j", g=G)
                gmax = spool.tile([P, BB, G], f32, tag="gmax")
                nc.vector.tensor_reduce(
                    gmax[:], e4, axis=X, op=mybir.AluOpType.max
                )
                gsort = spool.tile([P, BB, 8], f32, tag="gsort")
                for b in range(BB):
                    nc.vector.max(gsort[:, b, :], gmax[:, b, :])
                gmask = spool.tile([P, BB, G], f32, tag="gmask")
                nc.vector.tensor_tensor(
                    gmask[:],
                    gmax[:],
                    gsort[:, :, TG - 1 : TG].broadcast_to([P, BB, G]),
                    mybir.AluOpType.is_ge,
                )
                me = spool.tile([P, BB, E], f32, tag="me")
                nc.vector.tensor_tensor(
                    me[:].rearrange("p b (g j) -> p b g j", g=G),
                    e4,
                    gmask[:].unsqueeze(3).broadcast_to([P, BB, G, PG]),
                    mybir.AluOpType.mult,
                )
                t8 = spool.tile([P, BB, 8], f32, tag="t8")
                for b in range(BB):
                    nc.vector.max(t8[:, b, :], me[:, b, :])
                ssum = spool.tile([P, BB], f32, tag="ssum")
                nc.vector.tensor_reduce(
                    ssum[:], t8[:, :, 0:TK], axis=X, op=mybir.AluOpType.add
                )
                rec = spool.tile([P, BB], f32, tag="rec")
                nc.vector.reciprocal(rec[:], ssum[:])
                ow = opool.tile([P, BB, TK], f32, tag="ow")
                nc.vector.tensor_tensor(
                    ow[:],
                    t8[:, :, 0:TK],
                    rec[:].unsqueeze(2).broadcast_to([P, BB, TK]),
                    mybir.AluOpType.mult,
                )
                nc.sync.dma_start(out=our[:, m, b0 : b0 + nb], in_=ow[:])

            # two-stage software pipeline: transposes/copies of megatile m
            # are issued alongside the matmuls/topk of megatile m-1 so the
            # PE never waits on PSUM->SBUF copies of the tile it multiplies.
            prev = None
            w_done = False
            warm_pe()
            for _r in range(repeat):
                for m in range(NM):
                    cur = (m, *load_and_transpose(m))
                    if not w_done:
                        setup_w()
                        w_done = True
                    if prev is not None:
                        compute(*prev)
                    prev = cur
            if prev is not None:
                # split the final megatile so its top-k overlaps the second
                # half-chain instead of serializing after the last matmul
                m_l, xh_l, xl_l = prev
                compute(m_l, xh_l, xl_l, 0, MEGA // 2)
                compute(m_l, xh_l, xl_l, MEGA // 2, MEGA // 2)

    nc.compile()
    return nc


_NC_CACHE = {}


def _get_nc(t_core):
    if t_core not in _NC_CACHE:
        _NC_CACHE[t_core] = build_nc(t_core)
    return _NC_CACHE[t_core]


def run_sharded(flat_x, w, trace=False, **kw):
    """flat_x: [T, H] f32. Returns ([T, 6] f32, BassKernelResults)."""
    from concourse.bass_utils import run_bass_kernel_spmd

    T = flat_x.shape[0]
    tc = T // N_CORES
    nc = _get_nc(tc)
    in_maps = [
        {"x": np.ascontiguousarray(flat_x[i * tc : (i + 1) * tc]), "w": w}
        for i in range(N_CORES)
    ]
    res = run_bass_kernel_spmd(nc, in_maps, list(range(N_CORES)), trace=trace, **kw)
    outs = [np.asarray(res.results[i]["out"]) for i in range(N_CORES)]
    return np.concatenate(outs, axis=0), res


def kernel(hidden_states, kernel):
    hs = np.asarray(hidden_states, dtype=np.float32)
    w = np.ascontiguousarray(np.asarray(kernel, dtype=np.float32))
    B, S, Hh = hs.shape
    flat = np.ascontiguousarray(hs.reshape(B * S, Hh))
    out, _ = run_sharded(flat, w)
    return out
